# revision 4
# baseline (speedup 1.0000x reference)
"""Trainium2 Bass kernel for nn_MultiHeadAttention_63814624084186.

Reference computation (per batch sample b, fully independent across b):
  x: [512, 4096]  (C channels x N=64*64 pixels)
  qkv = w_qkv @ x            -> q,k,v each [512, 4096] (8 heads x 64 dims)
  scores = (q_h @ k_h^T)/8   -> [64, 64] per head   (channel-attention)
  attn = softmax(scores, -1)
  out_h = attn_h @ v_h       -> [64, 4096]
  y = w_out @ out + b_out    -> [512, 4096]
  y = groupnorm(y over all C,N) * gamma + beta

Sharding: pure data-parallel over batch: 16 samples / 8 cores = 2 per core.

Per-core dataflow (per batch):
  qT, kT computed GEMM-transposed ([N,512] layout) so the score GEMM can
  contract over N on the PE partition dim with no explicit transposes.
  Scores for head pairs are computed as [128,128] blocks (diag 64x64 blocks
  used).  attn^T is built block-diagonal [128,128] so attn@v runs with K=128.
"""

import numpy as np
from contextlib import ExitStack

import concourse.bass as bass
import concourse.tile as tile
from concourse import bacc, mybir
from concourse.bass_utils import run_bass_kernel_spmd
from concourse.masks import make_identity

F32 = mybir.dt.float32
BF16 = mybir.dt.bfloat16
AX = mybir.AxisListType
ALU = mybir.AluOpType
ACTF = mybir.ActivationFunctionType

B = 16          # global batch
C = 512         # channels
N = 4096        # pixels (64*64)
HW_SIDE = 64
NCORES = 8
PB = B // NCORES  # batches per core
P = 128
KC = C // P     # 4 channel chunks
NCH = N // P    # 32 pixel chunks of 128
NS = N // 512   # 8 pixel chunks of 512
NHP = 4         # head pairs
EPS = 1e-5

# matmul storage dtype per GEMM group: "bf16" | "f32r" | "f32"
CFG = dict(qk="bf16", v="bf16", av="bf16", out="bf16")


def _dt(kind):
    return BF16 if kind == "bf16" else F32


def _mm_ap(ap, kind):
    """AP to feed the tensor engine for a tile stored via _dt(kind)."""
    if kind == "f32r":
        return ap.bitcast(mybir.dt.float32r)
    return ap


def build_nc(cfg=CFG):
    nc = bacc.Bacc("TRN2", target_bir_lowering=False, debug=False,
                   num_devices=NCORES)

    x_d = nc.declare_dram_parameter("x", [PB, C, N], F32, isOutput=False)
    wq_d = nc.declare_dram_parameter("wq", [C, C], F32, isOutput=False)
    wk_d = nc.declare_dram_parameter("wk", [C, C], F32, isOutput=False)
    wv_d = nc.declare_dram_parameter("wv", [C, C], F32, isOutput=False)
    wo_d = nc.declare_dram_parameter("wo", [C, C], F32, isOutput=False)
    bias_d = nc.declare_dram_parameter("bvec", [C], F32, isOutput=False)
    gamma_d = nc.declare_dram_parameter("gamma", [C], F32, isOutput=False)
    beta_d = nc.declare_dram_parameter("beta", [C], F32, isOutput=False)
    out_d = nc.declare_dram_parameter("out", [PB, C, N], F32, isOutput=True)

    dt_qk, dt_v, dt_av, dt_out = (_dt(cfg[k]) for k in ("qk", "v", "av", "out"))

    with tile.TileContext(nc) as tc, ExitStack() as ctx:
        consts = ctx.enter_context(tc.tile_pool(name="consts", bufs=1))
        wstage = ctx.enter_context(tc.tile_pool(name="wstage", bufs=2))
        xstage = ctx.enter_context(tc.tile_pool(name="xstage", bufs=2))
        xpool = ctx.enter_context(tc.tile_pool(name="xpool", bufs=1))
        big = ctx.enter_context(tc.tile_pool(name="big", bufs=2))
        vpool = ctx.enter_context(tc.tile_pool(name="vpool", bufs=2))
        aopool = ctx.enter_context(tc.tile_pool(name="aopool", bufs=1))
        attn = ctx.enter_context(tc.tile_pool(name="attn", bufs=8))
        attnt = ctx.enter_context(tc.tile_pool(name="attnt", bufs=4))
        stats = ctx.enter_context(tc.tile_pool(name="stats", bufs=2))
        psmm = ctx.enter_context(tc.tile_pool(name="psmm", bufs=3, space="PSUM"))
        pssc = ctx.enter_context(tc.tile_pool(name="pssc", bufs=2, space="PSUM"))
        psst = ctx.enter_context(tc.tile_pool(name="psst", bufs=1, space="PSUM"))

        # ---- constants / weights (loaded once) ----
        def load_w(dram, dt):
            # dram [C, C] channel-major -> sbuf [128, KC, 512] (p = c % 128)
            t = consts.tile([P, KC, C], dt, tag=f"w_{dram.name}")
            src = dram.rearrange("(k p) o -> p k o", p=P)
            if dt == F32:
                nc.sync.dma_start(out=t, in_=src)
            else:
                s = wstage.tile([P, KC, C], F32, tag="wstage")
                nc.sync.dma_start(out=s, in_=src)
                nc.vector.tensor_copy(out=t, in_=s)
            return t

        wq_sb = load_w(wq_d, dt_qk)
        wk_sb = load_w(wk_d, dt_qk)
        wv_sb = load_w(wv_d, dt_v)
        wo_sb = load_w(wo_d, dt_out)

        bias_sb = consts.tile([P, KC], F32, tag="bias")
        nc.sync.dma_start(out=bias_sb, in_=bias_d.rearrange("(k p) -> p k", p=P))
        gamma_sb = consts.tile([P, KC], F32, tag="gamma")
        nc.sync.dma_start(out=gamma_sb, in_=gamma_d.rearrange("(k p) -> p k", p=P))
        beta_sb = consts.tile([P, KC], F32, tag="beta")
        nc.sync.dma_start(out=beta_sb, in_=beta_d.rearrange("(k p) -> p k", p=P))

        ident = consts.tile([P, P], dt_av, tag="ident")
        make_identity(nc, ident)

        eps_sb = consts.tile([1, 1], F32, tag="eps")
        nc.vector.memset(eps_sb, EPS)
        ones_col = consts.tile([P, 1], F32, tag="ones_col")
        nc.vector.memset(ones_col, 1.0)
        ones_row = consts.tile([1, P], F32, tag="ones_row")
        nc.vector.memset(ones_row, 1.0)

        for b in range(PB):
            # ---- phase 0: load x (+cast) ----
            # x stored [128, KC, N]; partition p = channel c % 128
            x_sb = xpool.tile([P, KC, N], dt_qk, tag="x")
            for k in range(KC):
                for h in range(2):
                    xs = xstage.tile([P, N // 2], F32, tag="xs")
                    nc.sync.dma_start(
                        out=xs,
                        in_=x_d[b, k * P:(k + 1) * P,
                                h * (N // 2):(h + 1) * (N // 2)])
                    nc.vector.tensor_copy(
                        out=x_sb[:, k, h * (N // 2):(h + 1) * (N // 2)], in_=xs)

            # ---- phase 1: qT, kT  ([N, 512] layouts) ----
            qT = big.tile([P, NCH, C], dt_qk, tag="big")
            kT = big.tile([P, NCH, C], dt_qk, tag="big")
            for dst, w in ((qT, wq_sb), (kT, wk_sb)):
                for i in range(NCH):
                    ps = psmm.tile([P, C], F32, tag="psmm")
                    for k in range(KC):
                        nc.tensor.matmul(
                            ps,
                            lhsT=_mm_ap(x_sb[:, k, i * P:(i + 1) * P], cfg["qk"]),
                            rhs=_mm_ap(w[:, k, :], cfg["qk"]),
                            start=(k == 0), stop=(k == KC - 1))
                    nc.scalar.copy(out=dst[:, i, :], in_=ps)

            # ---- phase 2: scores per head pair (diag blocks of [128,128]) ----
            attnT_tiles = []
            for hp in range(NHP):
                sc_ps = pssc.tile([P, P], F32, tag="pssc")
                cl = slice(hp * P, (hp + 1) * P)
                for i in range(NCH):
                    nc.tensor.matmul(
                        sc_ps,
                        lhsT=_mm_ap(qT[:, i, cl], cfg["qk"]),
                        rhs=_mm_ap(kT[:, i, cl], cfg["qk"]),
                        start=(i == 0), stop=(i == NCH - 1))

                # ---- phase 3: softmax over the two diagonal 64x64 blocks ----
                a_sc = attn.tile([P, 64], F32, tag="a_sc")
                nc.vector.tensor_copy(out=a_sc[0:64, :], in_=sc_ps[0:64, 0:64])
                nc.vector.tensor_copy(out=a_sc[64:P, :], in_=sc_ps[64:P, 64:P])
                mx = attn.tile([P, 1], F32, tag="mx")
                nc.vector.reduce_max(out=mx, in_=a_sc, axis=AX.X)
                nmx = attn.tile([P, 1], F32, tag="nmx")
                nc.vector.tensor_scalar_mul(nmx, mx, -0.125)
                a_e = attn.tile([P, 64], F32, tag="a_e")
                nc.scalar.activation(out=a_e, in_=a_sc, func=ACTF.Exp,
                                     bias=nmx, scale=0.125)
                sm = attn.tile([P, 1], F32, tag="sm")
                nc.vector.reduce_sum(out=sm, in_=a_e, axis=AX.X)
                rs = attn.tile([P, 1], F32, tag="rs")
                nc.vector.reciprocal(out=rs, in_=sm)
                a_mm = attn.tile([P, 64], dt_av, tag="a_mm")
                nc.vector.tensor_scalar_mul(a_mm, a_e, rs)

                # block-diagonal attn^T [128,128]
                at = attnt.tile([P, P], dt_av, tag="attnT")
                nc.gpsimd.memset(at, 0.0)
                attnT_tiles.append((at, a_mm))

            # transposes emitted later (after some v work) to keep PE busy
            # while softmax runs on DVE/ACT.
            def emit_transpose(hp):
                at, a_mm = attnT_tiles[hp]
                pt = psmm.tile([P, 64], dt_av, tag="psmm")
                nc.tensor.transpose(pt[0:64, :], a_mm[0:64, :], ident[0:64, 0:64])
                nc.tensor.transpose(pt[64:P, :], a_mm[64:P, :], ident[64:P, 64:P])
                nc.vector.tensor_copy(out=at[0:64, 0:64], in_=pt[0:64, :])
                nc.vector.tensor_copy(out=at[64:P, 64:P], in_=pt[64:P, :])

            # ---- phase 4: v GEMM + attn @ v, per head pair ----
            v_tiles = {}

            def emit_v(hp):
                v_p = vpool.tile([P, NS, 512], dt_v, tag="v")
                cl = slice(hp * P, (hp + 1) * P)
                for ns in range(NS):
                    ps = psmm.tile([P, 512], F32, tag="psmm")
                    for k in range(KC):
                        nc.tensor.matmul(
                            ps,
                            lhsT=_mm_ap(wv_sb[:, k, cl], cfg["v"]),
                            rhs=_mm_ap(x_sb[:, k, ns * 512:(ns + 1) * 512],
                                       cfg["v"]),
                            start=(k == 0), stop=(k == KC - 1))
                    nc.vector.tensor_copy(out=v_p[:, ns, :], in_=ps)
                v_tiles[hp] = v_p

            ao = aopool.tile([P, KC, N], dt_av, tag="ao")

            def emit_ao(hp):
                at, _ = attnT_tiles[hp]
                v_p = v_tiles[hp]
                for ns in range(NS):
                    ps = psmm.tile([P, 512], F32, tag="psmm")
                    nc.tensor.matmul(ps, lhsT=_mm_ap(at, cfg["av"]),
                                     rhs=_mm_ap(v_p[:, ns, :], cfg["av"]),
                                     start=True, stop=True)
                    nc.vector.tensor_copy(out=ao[:, hp, ns * 512:(ns + 1) * 512],
                                          in_=ps)

            emit_v(0)
            emit_v(1)
            for hp in range(NHP):
                emit_transpose(hp)
            emit_ao(0)
            emit_v(2)
            emit_ao(1)
            emit_v(3)
            emit_ao(2)
            emit_ao(3)

            # ---- phase 5: out projection + bias; bn stats on the fly ----
            y_lo = big.tile([P, 2, N], F32, tag="big")
            y_hi = big.tile([P, 2, N], F32, tag="big")
            st = stats.tile([P, KC, NS, 6], F32, tag="bnstats")
            for m in range(KC):
                yt = y_lo if m < 2 else y_hi
                mi = m % 2
                for ns in range(NS):
                    ps = psmm.tile([P, 512], F32, tag="psmm")
                    for k in range(KC):
                        nc.tensor.matmul(
                            ps,
                            lhsT=_mm_ap(wo_sb[:, k, m * P:(m + 1) * P],
                                        cfg["out"]),
                            rhs=_mm_ap(ao[:, k, ns * 512:(ns + 1) * 512],
                                       cfg["out"]),
                            start=(k == 0), stop=(k == KC - 1))
                    ysl = yt[:, mi, ns * 512:(ns + 1) * 512]
                    nc.scalar.add(out=ysl, in_=ps, add=bias_sb[:, m:m + 1])
                    nc.vector.bn_stats(out=st[:, m, ns, :], in_=ysl)

            # ---- phase 6: global mean/var across all 512*4096 elements ----
            mv = stats.tile([P, KC, 2], F32, tag="mv")
            for m in range(KC):
                nc.vector.bn_aggr(out=mv[:, m, :], in_=st[:, m])
            # S[p, stat, m]: stat 0 = mean, 1 = var, 2 = mean^2
            s_t = stats.tile([P, 3, KC], F32, tag="s_t")
            nc.vector.tensor_copy(out=s_t[:, 0, :], in_=mv[:, :, 0])
            nc.vector.tensor_copy(out=s_t[:, 1, :], in_=mv[:, :, 1])
            nc.vector.tensor_mul(s_t[:, 2, :], mv[:, :, 0], mv[:, :, 0])
            # sum over partitions via PE (ones vector)
            pstat = psst.tile([1, 3, KC], F32, tag="psst")
            nc.tensor.matmul(pstat, lhsT=ones_col, rhs=s_t,
                             start=True, stop=True)
            red = stats.tile([1, 3], F32, tag="red")
            nc.vector.reduce_sum(out=red, in_=pstat, axis=AX.X)
            e3 = stats.tile([1, 3], F32, tag="e3")
            nc.vector.tensor_scalar_mul(e3, red, 1.0 / C)
            m2 = stats.tile([1, 1], F32, tag="m2")
            nc.vector.tensor_mul(m2, e3[:, 0:1], e3[:, 0:1])
            var = stats.tile([1, 1], F32, tag="var")
            nc.vector.tensor_add(var, e3[:, 1:2], e3[:, 2:3])
            nc.vector.tensor_sub(var, var, m2)
            std = stats.tile([1, 1], F32, tag="std")
            nc.scalar.activation(out=std, in_=var, func=ACTF.Sqrt,
                                 bias=eps_sb, scale=1.0)
            rstd = stats.tile([1, 1], F32, tag="rstd")
            nc.vector.reciprocal(out=rstd, in_=std)
            sc2 = stats.tile([1, 2], F32, tag="sc2")
            nc.vector.tensor_copy(out=sc2[:, 0:1], in_=e3[:, 0:1])
            nc.vector.tensor_copy(out=sc2[:, 1:2], in_=rstd)
            bc_ps = psst.tile([P, 2], F32, tag="psbc")
            nc.tensor.matmul(bc_ps, lhsT=ones_row, rhs=sc2,
                             start=True, stop=True)
            bc = stats.tile([P, 2], F32, tag="bc")
            nc.vector.tensor_copy(out=bc, in_=bc_ps)
            # per-channel scale s = gamma * rstd ; shift t = beta - mean * s
            s_ch = stats.tile([P, KC], F32, tag="s_ch")
            nc.vector.tensor_scalar_mul(s_ch, gamma_sb, bc[:, 1:2])
            t_ch = stats.tile([P, KC], F32, tag="t_ch")
            nc.vector.tensor_scalar_mul(t_ch, s_ch, bc[:, 0:1])
            nc.vector.tensor_sub(t_ch, beta_sb, t_ch)

            # ---- phase 7: apply + write out ----
            for m in range(KC):
                yt = y_lo if m < 2 else y_hi
                mi = m % 2
                nc.vector.tensor_scalar(
                    out=yt[:, mi, :], in0=yt[:, mi, :],
                    scalar1=s_ch[:, m:m + 1], scalar2=t_ch[:, m:m + 1],
                    op0=ALU.mult, op1=ALU.add)
                nc.sync.dma_start(out=out_d[b, m * P:(m + 1) * P, :],
                                  in_=yt[:, mi, :])

    nc.finalize()
    return nc


_NC_CACHE = {}


def _get_nc(cfg_key=None):
    key = tuple(sorted(CFG.items())) if cfg_key is None else cfg_key
    if key not in _NC_CACHE:
        _NC_CACHE[key] = build_nc(dict(key))
    return _NC_CACHE[key]


def _make_in_maps(x, w_qkv, w_out, b_out, gamma, beta):
    xr = np.ascontiguousarray(np.asarray(x, dtype=np.float32).reshape(B, C, N))
    w_qkv = np.asarray(w_qkv, dtype=np.float32)
    wq = np.ascontiguousarray(w_qkv[0:C].T)
    wk = np.ascontiguousarray(w_qkv[C:2 * C].T)
    wv = np.ascontiguousarray(w_qkv[2 * C:3 * C].T)
    wo = np.ascontiguousarray(np.asarray(w_out, dtype=np.float32).T)
    b_out = np.ascontiguousarray(np.asarray(b_out, dtype=np.float32))
    gamma = np.ascontiguousarray(np.asarray(gamma, dtype=np.float32))
    beta = np.ascontiguousarray(np.asarray(beta, dtype=np.float32))
    return [
        dict(x=np.ascontiguousarray(xr[c * PB:(c + 1) * PB]),
             wq=wq, wk=wk, wv=wv, wo=wo,
             bvec=b_out, gamma=gamma, beta=beta)
        for c in range(NCORES)
    ]


def _run(inputs, trace=False, trace_kwargs=None):
    nc = _get_nc()
    in_maps = _make_in_maps(**inputs)
    res = run_bass_kernel_spmd(nc, in_maps, core_ids=list(range(NCORES)),
                               trace=trace, **(trace_kwargs or {}))
    out = np.concatenate([res.results[c]["out"] for c in range(NCORES)], axis=0)
    return out.reshape(B, C, HW_SIDE, HW_SIDE), res


def kernel(x, w_qkv, w_out, b_out, gamma, beta):
    out, _ = _run(dict(x=x, w_qkv=w_qkv, w_out=w_out, b_out=b_out,
                       gamma=gamma, beta=beta))
    return out


# revision 8
# speedup vs baseline: 1.0451x; 1.0451x over previous
"""Trainium2 Bass kernel for nn_MultiHeadAttention_63814624084186.

Reference computation (per batch sample b, fully independent across b):
  x: [512, 4096]  (C channels x N=64*64 pixels)
  qkv = w_qkv @ x            -> q,k,v each [512, 4096] (8 heads x 64 dims)
  scores = (q_h @ k_h^T)/8   -> [64, 64] per head   (channel-attention)
  attn = softmax(scores, -1)
  out_h = attn_h @ v_h       -> [64, 4096]
  y = w_out @ out + b_out    -> [512, 4096]
  y = groupnorm(y over all C,N) * gamma + beta

Sharding: pure data-parallel over batch: 16 samples / 8 cores = 2 per core.

Design notes:
  - q/k/v GEMMs run in float32r (tf32-class precision at bf16-like speed
    for N=512).  x and w_q/w_k/w_v are DMA'd straight into float32r tiles
    (PE rounds on read; verified on HW).
  - phase 1+2 are n-blocked (8 blocks of 512 pixels): per block we DMA an
    x block (one DMA per channel chunk), compute qT/kT blocks ([N,512]
    layout via GEMM "transpose": lhsT = x block), accumulate scores into
    4 persistent PSUM banks, and compute v for the block.
  - scores/attn@v/out-proj run in bf16 (error contribution ~3e-3).
  - GroupNorm: bn_stats per PSUM tile (bias folded into the cross-
    partition combine), cross-partition reduce via ones-matmul.
  - The two batches are emitted interleaved:
    A(0) B(0) A(1) tail(0) B(1) tail(1), where A = blocked qkv+scores,
    B = softmax/attn@v/out-proj/bn_stats, tail = stat combine+apply+store.
    This hides batch 0's epilogue fully under batch 1's compute and keeps
    the PE queue free of stat matmuls between batches.
  - Weights arrive host-prearranged as [128, KC, C] so weight DMAs are
    contiguous per partition (few descriptors, fast issue).
"""

import numpy as np
from contextlib import ExitStack

import concourse.bass as bass
import concourse.tile as tile
from concourse import bacc, mybir
from concourse.bass_utils import run_bass_kernel_spmd
from concourse.masks import make_identity

F32 = mybir.dt.float32
F32R = mybir.dt.float32r
BF16 = mybir.dt.bfloat16
AX = mybir.AxisListType
ALU = mybir.AluOpType
ACTF = mybir.ActivationFunctionType

B = 16          # global batch
C = 512         # channels
N = 4096        # pixels (64*64)
HW_SIDE = 64
NCORES = 8
PB = B // NCORES  # batches per core
P = 128
KC = C // P     # 4 channel chunks
NB = 8          # n blocks of 512
NBI = 4         # 128-chunks per n block
NS = N // 512   # 8 pixel chunks of 512
NHP = 4         # head pairs
EPS = 1e-5


def build_nc():
    nc = bacc.Bacc("TRN2", target_bir_lowering=False, debug=False,
                   num_devices=NCORES)

    x_d = nc.declare_dram_parameter("x", [PB, C, N], F32, isOutput=False)
    wq_d = nc.declare_dram_parameter("wq", [P, KC, C], F32, isOutput=False)
    wk_d = nc.declare_dram_parameter("wk", [P, KC, C], F32, isOutput=False)
    wv_d = nc.declare_dram_parameter("wv", [P, KC, C], F32, isOutput=False)
    wo_d = nc.declare_dram_parameter("wo", [P, KC, C], F32, isOutput=False)
    bias_d = nc.declare_dram_parameter("bvec", [P, KC], F32, isOutput=False)
    gamma_d = nc.declare_dram_parameter("gamma", [P, KC], F32, isOutput=False)
    beta_d = nc.declare_dram_parameter("beta", [P, KC], F32, isOutput=False)
    out_d = nc.declare_dram_parameter("out", [PB, C, N], F32, isOutput=True)

    with tile.TileContext(nc) as tc, ExitStack() as ctx:
        consts = ctx.enter_context(tc.tile_pool(name="consts", bufs=1))
        xpool = ctx.enter_context(tc.tile_pool(name="xpool", bufs=3))
        qkpool = ctx.enter_context(tc.tile_pool(name="qkpool", bufs=4))
        vpool = ctx.enter_context(tc.tile_pool(name="vpool", bufs=1))
        aopool = ctx.enter_context(tc.tile_pool(name="aopool", bufs=1))
        ypool = ctx.enter_context(tc.tile_pool(name="ypool", bufs=2))
        attn = ctx.enter_context(tc.tile_pool(name="attn", bufs=8))
        attnt = ctx.enter_context(tc.tile_pool(name="attnt", bufs=4))
        stats = ctx.enter_context(tc.tile_pool(name="stats", bufs=2))
        psmm = ctx.enter_context(tc.tile_pool(name="psmm", bufs=4, space="PSUM"))
        pssc = ctx.enter_context(tc.tile_pool(name="pssc", bufs=4, space="PSUM"))

        # ---- constants / weights (loaded once; host-prearranged) ----
        def load_w_f32r(dram):
            t = consts.tile([P, KC, C], F32R, tag=f"w_{dram.name}")
            nc.gpsimd.dma_start(out=t, in_=dram[:, :, :].bitcast(F32R))
            return t

        wq_sb = load_w_f32r(wq_d)
        wk_sb = load_w_f32r(wk_d)
        wv_sb = load_w_f32r(wv_d)

        wo_sb = consts.tile([P, KC, C], BF16, tag="w_wo")
        wo_stage = xpool.tile([P, KC, C], F32, tag="xblk")
        nc.gpsimd.dma_start(out=wo_stage, in_=wo_d[:, :, :])
        nc.vector.tensor_copy(out=wo_sb, in_=wo_stage)

        bias_sb = consts.tile([P, KC], F32, tag="bias")
        nc.gpsimd.dma_start(out=bias_sb, in_=bias_d[:, :])
        gamma_sb = consts.tile([P, KC], F32, tag="gamma")
        nc.gpsimd.dma_start(out=gamma_sb, in_=gamma_d[:, :])
        beta_sb = consts.tile([P, KC], F32, tag="beta")
        nc.gpsimd.dma_start(out=beta_sb, in_=beta_d[:, :])

        ident = consts.tile([P, P], BF16, tag="ident")
        make_identity(nc, ident)

        eps_sb = consts.tile([1, 1], F32, tag="eps")
        nc.vector.memset(eps_sb, EPS)
        ones_col = consts.tile([P, 1], F32, tag="ones_col")
        nc.vector.memset(ones_col, 1.0)
        ones_row = consts.tile([1, P], F32, tag="ones_row")
        nc.vector.memset(ones_row, 1.0)

        # per-batch state carried between emission stages
        st_v = {}
        st_sc = {}
        st_y = {}
        st_stats = {}

        def emit_A(b):
            """n-blocked qT/kT GEMMs, score accumulation, v GEMM."""
            v_sb = vpool.tile([P, NHP, N], BF16, tag="v", name=f"v_{b}")
            sc_ps = [pssc.tile([P, P], F32, tag="pssc", name=f"sc_{b}_{hp}")
                     for hp in range(NHP)]
            st_v[b] = v_sb
            st_sc[b] = sc_ps
            for j in range(NB):
                x_blk = xpool.tile([P, KC, 512], F32R, tag="xblk",
                                   name=f"x_{b}_{j}")
                for k in range(KC):
                    nc.sync.dma_start(
                        out=x_blk[:, k, :],
                        in_=x_d[b, k * P:(k + 1) * P, j * 512:(j + 1) * 512]
                            .bitcast(F32R))

                qT_blk = qkpool.tile([P, NBI, C], BF16, tag="qk",
                                     name=f"qT_{b}_{j}")
                kT_blk = qkpool.tile([P, NBI, C], BF16, tag="qk",
                                     name=f"kT_{b}_{j}")
                for dst, w in ((qT_blk, wq_sb), (kT_blk, wk_sb)):
                    for i in range(NBI):
                        ps = psmm.tile([P, C], F32, tag="psmm")
                        for k in range(KC):
                            nc.tensor.matmul(
                                ps,
                                lhsT=x_blk[:, k, i * P:(i + 1) * P],
                                rhs=w[:, k, :],
                                start=(k == 0), stop=(k == KC - 1))
                        nc.scalar.copy(out=dst[:, i, :], in_=ps)

                for hp in range(NHP):
                    cl = slice(hp * P, (hp + 1) * P)
                    for i in range(NBI):
                        nc.tensor.matmul(
                            sc_ps[hp],
                            lhsT=qT_blk[:, i, cl],
                            rhs=kT_blk[:, i, cl],
                            start=(j == 0 and i == 0),
                            stop=(j == NB - 1 and i == NBI - 1),
                            skip_group_check=True)

                for hp in range(NHP):
                    cl = slice(hp * P, (hp + 1) * P)
                    ps = psmm.tile([P, 512], F32, tag="psmm")
                    for k in range(KC):
                        nc.tensor.matmul(
                            ps,
                            lhsT=wv_sb[:, k, cl],
                            rhs=x_blk[:, k, :],
                            start=(k == 0), stop=(k == KC - 1))
                    nc.vector.tensor_copy(
                        out=v_sb[:, hp, j * 512:(j + 1) * 512], in_=ps)

        def emit_B(b):
            """softmax, attn transposes, attn@v, out projection, bn_stats."""
            v_sb = st_v[b]
            sc_ps = st_sc[b]
            attnT_tiles = []
            for hp in range(NHP):
                a_sc = attn.tile([P, 64], F32, tag="a_sc")
                nc.vector.tensor_copy(out=a_sc[0:64, :], in_=sc_ps[hp][0:64, 0:64])
                nc.vector.tensor_copy(out=a_sc[64:P, :], in_=sc_ps[hp][64:P, 64:P])
                mx = attn.tile([P, 1], F32, tag="mx")
                nc.vector.reduce_max(out=mx, in_=a_sc, axis=AX.X)
                nmx = attn.tile([P, 1], F32, tag="nmx")
                nc.vector.tensor_scalar_mul(nmx, mx, -0.125)
                a_e = attn.tile([P, 64], F32, tag="a_e")
                nc.scalar.activation(out=a_e, in_=a_sc, func=ACTF.Exp,
                                     bias=nmx, scale=0.125)
                sm = attn.tile([P, 1], F32, tag="sm")
                nc.vector.reduce_sum(out=sm, in_=a_e, axis=AX.X)
                rs = attn.tile([P, 1], F32, tag="rs")
                nc.vector.reciprocal(out=rs, in_=sm)
                a_mm = attn.tile([P, 64], BF16, tag="a_mm")
                nc.vector.tensor_scalar_mul(a_mm, a_e, rs)
                at = attnt.tile([P, P], BF16, tag="attnT", name=f"at_{b}_{hp}")
                nc.gpsimd.memset(at, 0.0)
                attnT_tiles.append((at, a_mm))

            for hp in range(NHP):
                at, a_mm = attnT_tiles[hp]
                pt = psmm.tile([P, 64], BF16, tag="psmm")
                nc.tensor.transpose(pt[0:64, :], a_mm[0:64, :], ident[0:64, 0:64])
                nc.tensor.transpose(pt[64:P, :], a_mm[64:P, :], ident[64:P, 64:P])
                nc.vector.tensor_copy(out=at[0:64, 0:64], in_=pt[0:64, :])
                nc.vector.tensor_copy(out=at[64:P, 64:P], in_=pt[64:P, :])

            ao = aopool.tile([P, KC, N], BF16, tag="ao", name=f"ao_{b}")
            for hp in range(NHP):
                at, _ = attnT_tiles[hp]
                for ns in range(NS):
                    ps = psmm.tile([P, 512], F32, tag="psmm")
                    nc.tensor.matmul(ps, lhsT=at,
                                     rhs=v_sb[:, hp, ns * 512:(ns + 1) * 512],
                                     start=True, stop=True)
                    nc.vector.tensor_copy(out=ao[:, hp, ns * 512:(ns + 1) * 512],
                                          in_=ps)

            y_lo = ypool.tile([P, 2, N], F32, tag="y", name=f"ylo_{b}")
            y_hi = ypool.tile([P, 2, N], F32, tag="y", name=f"yhi_{b}")
            st = stats.tile([P, KC, NS, 6], F32, tag="bnstats")
            st_y[b] = (y_lo, y_hi)
            st_stats[b] = st
            for m in range(KC):
                yt = y_lo if m < 2 else y_hi
                mi = m % 2
                for ns in range(NS):
                    ps = psmm.tile([P, 512], F32, tag="psmm")
                    for k in range(KC):
                        nc.tensor.matmul(
                            ps,
                            lhsT=wo_sb[:, k, m * P:(m + 1) * P],
                            rhs=ao[:, k, ns * 512:(ns + 1) * 512],
                            start=(k == 0), stop=(k == KC - 1))
                    # stats on pre-bias values (bias folded in below)
                    nc.vector.bn_stats(out=st[:, m, ns, :], in_=ps)
                    nc.scalar.add(out=yt[:, mi, ns * 512:(ns + 1) * 512],
                                  in_=ps, add=bias_sb[:, m:m + 1])

        def emit_tail(b):
            """global mean/var combine, normalization apply, writeout."""
            y_lo, y_hi = st_y[b]
            st = st_stats[b]
            mv = stats.tile([P, KC, 2], F32, tag="mv")
            for m in range(KC):
                nc.vector.bn_aggr(out=mv[:, m, :], in_=st[:, m])
            # S[p, stat, m]: 0 = mean+bias, 1 = var, 2 = (mean+bias)^2
            s_t = stats.tile([P, 3, KC], F32, tag="s_t")
            nc.vector.tensor_add(s_t[:, 0, :], mv[:, :, 0], bias_sb)
            nc.vector.tensor_copy(out=s_t[:, 1, :], in_=mv[:, :, 1])
            nc.vector.tensor_mul(s_t[:, 2, :], s_t[:, 0, :], s_t[:, 0, :])
            pstat = psmm.tile([1, 3, KC], F32, tag="psmm")
            nc.tensor.matmul(pstat, lhsT=ones_col, rhs=s_t,
                             start=True, stop=True)
            red = stats.tile([1, 3], F32, tag="red")
            nc.vector.reduce_sum(out=red, in_=pstat, axis=AX.X)
            e3 = stats.tile([1, 3], F32, tag="e3")
            nc.vector.tensor_scalar_mul(e3, red, 1.0 / C)
            m2 = stats.tile([1, 1], F32, tag="m2")
            nc.vector.tensor_mul(m2, e3[:, 0:1], e3[:, 0:1])
            var = stats.tile([1, 1], F32, tag="var")
            nc.vector.tensor_add(var, e3[:, 1:2], e3[:, 2:3])
            nc.vector.tensor_sub(var, var, m2)
            std = stats.tile([1, 1], F32, tag="std")
            nc.scalar.activation(out=std, in_=var, func=ACTF.Sqrt,
                                 bias=eps_sb, scale=1.0)
            rstd = stats.tile([1, 1], F32, tag="rstd")
            nc.vector.reciprocal(out=rstd, in_=std)
            sc2 = stats.tile([1, 2], F32, tag="sc2")
            nc.vector.tensor_copy(out=sc2[:, 0:1], in_=e3[:, 0:1])
            nc.vector.tensor_copy(out=sc2[:, 1:2], in_=rstd)
            bc_ps = psmm.tile([P, 2], F32, tag="psmm")
            nc.tensor.matmul(bc_ps, lhsT=ones_row, rhs=sc2,
                             start=True, stop=True)
            # s = gamma * rstd ; t = beta - mean_total * s
            s_ch = stats.tile([P, KC], F32, tag="s_ch")
            nc.vector.tensor_scalar_mul(s_ch, gamma_sb, bc_ps[:, 1:2])
            t_ch = stats.tile([P, KC], F32, tag="t_ch")
            nc.vector.tensor_scalar_mul(t_ch, s_ch, bc_ps[:, 0:1])
            nc.vector.tensor_sub(t_ch, beta_sb, t_ch)

            for m in range(KC):
                yt = y_lo if m < 2 else y_hi
                mi = m % 2
                for h in range(2):
                    sl = slice(h * (N // 2), (h + 1) * (N // 2))
                    if m % 2 == 0:
                        nc.vector.tensor_scalar(
                            out=yt[:, mi, sl], in0=yt[:, mi, sl],
                            scalar1=s_ch[:, m:m + 1], scalar2=t_ch[:, m:m + 1],
                            op0=ALU.mult, op1=ALU.add)
                    else:
                        nc.scalar.activation(
                            out=yt[:, mi, sl], in_=yt[:, mi, sl],
                            func=ACTF.Identity,
                            bias=t_ch[:, m:m + 1], scale=s_ch[:, m:m + 1])
                    nc.sync.dma_start(out=out_d[b, m * P:(m + 1) * P, sl],
                                      in_=yt[:, mi, sl])

        emit_A(0)
        emit_B(0)
        emit_A(1)
        emit_tail(0)
        emit_B(1)
        emit_tail(1)

    nc.finalize()
    return nc


_NC_CACHE = {}


def _get_nc():
    if "nc" not in _NC_CACHE:
        _NC_CACHE["nc"] = build_nc()
    return _NC_CACHE["nc"]


def _prep_w(w):
    # [C_in, C_out] -> [128, KC, C_out] with c_in = k*128 + p, contiguous
    return np.ascontiguousarray(w.reshape(KC, P, C).transpose(1, 0, 2))


def _prep_vec(v):
    # [C] -> [128, KC] with c = k*128 + p
    return np.ascontiguousarray(v.reshape(KC, P).T)


def _make_in_maps(x, w_qkv, w_out, b_out, gamma, beta):
    xr = np.ascontiguousarray(np.asarray(x, dtype=np.float32).reshape(B, C, N))
    w_qkv = np.asarray(w_qkv, dtype=np.float32)
    wq = _prep_w(np.ascontiguousarray(w_qkv[0:C].T))
    wk = _prep_w(np.ascontiguousarray(w_qkv[C:2 * C].T))
    wv = _prep_w(np.ascontiguousarray(w_qkv[2 * C:3 * C].T))
    wo = _prep_w(np.ascontiguousarray(np.asarray(w_out, dtype=np.float32).T))
    bvec = _prep_vec(np.asarray(b_out, dtype=np.float32))
    gam = _prep_vec(np.asarray(gamma, dtype=np.float32))
    bet = _prep_vec(np.asarray(beta, dtype=np.float32))
    return [
        dict(x=np.ascontiguousarray(xr[c * PB:(c + 1) * PB]),
             wq=wq, wk=wk, wv=wv, wo=wo,
             bvec=bvec, gamma=gam, beta=bet)
        for c in range(NCORES)
    ]


def _run(inputs, trace=False, trace_kwargs=None):
    nc = _get_nc()
    in_maps = _make_in_maps(**inputs)
    res = run_bass_kernel_spmd(nc, in_maps, core_ids=list(range(NCORES)),
                               trace=trace, **(trace_kwargs or {}))
    out = np.concatenate([res.results[c]["out"] for c in range(NCORES)], axis=0)
    return out.reshape(B, C, HW_SIDE, HW_SIDE), res


def kernel(x, w_qkv, w_out, b_out, gamma, beta):
    out, _ = _run(dict(x=x, w_qkv=w_qkv, w_out=w_out, b_out=b_out,
                       gamma=gamma, beta=beta))
    return out


# revision 10
# speedup vs baseline: 1.0560x; 1.0104x over previous
"""Trainium2 Bass kernel for nn_MultiHeadAttention_63814624084186.

Reference computation (per batch sample b, fully independent across b):
  x: [512, 4096]  (C channels x N=64*64 pixels)
  qkv = w_qkv @ x            -> q,k,v each [512, 4096] (8 heads x 64 dims)
  scores = (q_h @ k_h^T)/8   -> [64, 64] per head   (channel-attention)
  attn = softmax(scores, -1)
  out_h = attn_h @ v_h       -> [64, 4096]
  y = w_out @ out + b_out    -> [512, 4096]
  y = groupnorm(y over all C,N) * gamma + beta

Sharding: pure data-parallel over batch: 16 samples / 8 cores = 2 per core.

Design notes:
  - q/k/v GEMMs run in float32r (tf32-class precision at bf16-like speed
    for N=512).  x and w_q/w_k/w_v are DMA'd straight into float32r tiles
    (PE rounds on read; verified on HW).
  - phase 1+2 are n-blocked (8 blocks of 512 pixels): per block we DMA an
    x block (one DMA per channel chunk), compute qT/kT blocks ([N,512]
    layout via GEMM "transpose": lhsT = x block), accumulate scores into
    4 persistent PSUM banks, and compute v for the block.
  - scores/attn@v/out-proj run in bf16 (error contribution ~3e-3).
  - GroupNorm: bn_stats per PSUM tile (bias folded into the cross-
    partition combine), cross-partition reduce via ones-matmul.
  - The two batches are emitted interleaved:
    A(0) B(0) A(1) tail(0) B(1) tail(1), where A = blocked qkv+scores,
    B = softmax/attn@v/out-proj/bn_stats, tail = stat combine+apply+store.
    This hides batch 0's epilogue fully under batch 1's compute and keeps
    the PE queue free of stat matmuls between batches.
  - Weights arrive host-prearranged as [128, KC, C] so weight DMAs are
    contiguous per partition (few descriptors, fast issue).
"""

import numpy as np
from contextlib import ExitStack

import concourse.bass as bass
import concourse.tile as tile
from concourse import bacc, mybir
from concourse.bass_utils import run_bass_kernel_spmd
from concourse.masks import make_identity

F32 = mybir.dt.float32
F32R = mybir.dt.float32r
BF16 = mybir.dt.bfloat16
AX = mybir.AxisListType
ALU = mybir.AluOpType
ACTF = mybir.ActivationFunctionType

B = 16          # global batch
C = 512         # channels
N = 4096        # pixels (64*64)
HW_SIDE = 64
NCORES = 8
PB = B // NCORES  # batches per core
P = 128
KC = C // P     # 4 channel chunks
NB = 8          # n blocks of 512
NBI = 4         # 128-chunks per n block
NS = N // 512   # 8 pixel chunks of 512
NHP = 4         # head pairs
EPS = 1e-5


def build_nc():
    nc = bacc.Bacc("TRN2", target_bir_lowering=False, debug=False,
                   num_devices=NCORES)

    x_d = nc.declare_dram_parameter("x", [PB, NB, P, KC * 512], F32, isOutput=False)
    wq_d = nc.declare_dram_parameter("wq", [P, KC, C], F32, isOutput=False)
    wk_d = nc.declare_dram_parameter("wk", [P, KC, C], F32, isOutput=False)
    wv_d = nc.declare_dram_parameter("wv", [P, KC, C], F32, isOutput=False)
    wo_d = nc.declare_dram_parameter("wo", [P, KC, C], F32, isOutput=False)
    bias_d = nc.declare_dram_parameter("bvec", [P, KC], F32, isOutput=False)
    gamma_d = nc.declare_dram_parameter("gamma", [P, KC], F32, isOutput=False)
    beta_d = nc.declare_dram_parameter("beta", [P, KC], F32, isOutput=False)
    out_d = nc.declare_dram_parameter("out", [PB, C, N], F32, isOutput=True)

    with tile.TileContext(nc) as tc, ExitStack() as ctx:
        consts = ctx.enter_context(tc.tile_pool(name="consts", bufs=1))
        xpool = ctx.enter_context(tc.tile_pool(name="xpool", bufs=3))
        qkpool = ctx.enter_context(tc.tile_pool(name="qkpool", bufs=4))
        vpool = ctx.enter_context(tc.tile_pool(name="vpool", bufs=1))
        aopool = ctx.enter_context(tc.tile_pool(name="aopool", bufs=1))
        ypool = ctx.enter_context(tc.tile_pool(name="ypool", bufs=2))
        attn = ctx.enter_context(tc.tile_pool(name="attn", bufs=8))
        attnt = ctx.enter_context(tc.tile_pool(name="attnt", bufs=4))
        stats = ctx.enter_context(tc.tile_pool(name="stats", bufs=2))
        psmm = ctx.enter_context(tc.tile_pool(name="psmm", bufs=4, space="PSUM"))
        pssc = ctx.enter_context(tc.tile_pool(name="pssc", bufs=4, space="PSUM"))

        # ---- prefetch first x blocks before weights (lead-in) ----
        prefetched_x = {}
        for j in range(3):
            xt = xpool.tile([P, KC, 512], F32R, tag="xblk", name=f"x_0_{j}")
            nc.sync.dma_start(
                out=xt,
                in_=x_d[0, j].rearrange("p (k n) -> p k n", k=KC).bitcast(F32R))
            prefetched_x[j] = xt

        # ---- constants / weights (loaded once; host-prearranged) ----
        def load_w_f32r(dram):
            t = consts.tile([P, KC, C], F32R, tag=f"w_{dram.name}")
            for k in range(KC):
                nc.sync.dma_start(out=t[:, k, :], in_=dram[:, k, :].bitcast(F32R))
            return t

        wq_sb = load_w_f32r(wq_d)
        wk_sb = load_w_f32r(wk_d)
        wv_sb = load_w_f32r(wv_d)

        wo_sb = consts.tile([P, KC, C], BF16, tag="w_wo")
        wo_stage = xpool.tile([P, KC, C], F32, tag="xblk")
        nc.gpsimd.dma_start(out=wo_stage, in_=wo_d[:, :, :])
        nc.vector.tensor_copy(out=wo_sb, in_=wo_stage)

        bias_sb = consts.tile([P, KC], F32, tag="bias")
        nc.gpsimd.dma_start(out=bias_sb, in_=bias_d[:, :])
        gamma_sb = consts.tile([P, KC], F32, tag="gamma")
        nc.gpsimd.dma_start(out=gamma_sb, in_=gamma_d[:, :])
        beta_sb = consts.tile([P, KC], F32, tag="beta")
        nc.gpsimd.dma_start(out=beta_sb, in_=beta_d[:, :])

        ident = consts.tile([P, P], BF16, tag="ident")
        make_identity(nc, ident)

        eps_sb = consts.tile([1, 1], F32, tag="eps")
        nc.vector.memset(eps_sb, EPS)
        ones_col = consts.tile([P, 1], F32, tag="ones_col")
        nc.vector.memset(ones_col, 1.0)
        ones_row = consts.tile([1, P], F32, tag="ones_row")
        nc.vector.memset(ones_row, 1.0)

        # per-batch state carried between emission stages
        st_v = {}
        st_sc = {}
        st_y = {}
        st_stats = {}

        def emit_A(b):
            """n-blocked qT/kT GEMMs, score accumulation, v GEMM."""
            v_sb = vpool.tile([P, NHP, N], BF16, tag="v", name=f"v_{b}")
            sc_ps = [pssc.tile([P, P], F32, tag="pssc", name=f"sc_{b}_{hp}")
                     for hp in range(NHP)]
            st_v[b] = v_sb
            st_sc[b] = sc_ps
            for j in range(NB):
                if b == 0 and j in prefetched_x:
                    x_blk = prefetched_x[j]
                else:
                    x_blk = xpool.tile([P, KC, 512], F32R, tag="xblk",
                                       name=f"x_{b}_{j}")
                    nc.sync.dma_start(
                        out=x_blk,
                        in_=x_d[b, j].rearrange("p (k n) -> p k n", k=KC)
                            .bitcast(F32R))

                qT_blk = qkpool.tile([P, NBI, C], BF16, tag="qk",
                                     name=f"qT_{b}_{j}")
                kT_blk = qkpool.tile([P, NBI, C], BF16, tag="qk",
                                     name=f"kT_{b}_{j}")
                for dst, w in ((qT_blk, wq_sb), (kT_blk, wk_sb)):
                    for i in range(NBI):
                        ps = psmm.tile([P, C], F32, tag="psmm")
                        for k in range(KC):
                            nc.tensor.matmul(
                                ps,
                                lhsT=x_blk[:, k, i * P:(i + 1) * P],
                                rhs=w[:, k, :],
                                start=(k == 0), stop=(k == KC - 1))
                        nc.scalar.copy(out=dst[:, i, :], in_=ps)

                for hp in range(NHP):
                    cl = slice(hp * P, (hp + 1) * P)
                    for i in range(NBI):
                        nc.tensor.matmul(
                            sc_ps[hp],
                            lhsT=qT_blk[:, i, cl],
                            rhs=kT_blk[:, i, cl],
                            start=(j == 0 and i == 0),
                            stop=(j == NB - 1 and i == NBI - 1),
                            skip_group_check=True)

                for hp in range(NHP):
                    cl = slice(hp * P, (hp + 1) * P)
                    ps = psmm.tile([P, 512], F32, tag="psmm")
                    for k in range(KC):
                        nc.tensor.matmul(
                            ps,
                            lhsT=wv_sb[:, k, cl],
                            rhs=x_blk[:, k, :],
                            start=(k == 0), stop=(k == KC - 1))
                    nc.vector.tensor_copy(
                        out=v_sb[:, hp, j * 512:(j + 1) * 512], in_=ps)

        st_ao = {}

        def emit_Bhead(b):
            """softmax, attn transposes, attn@v."""
            v_sb = st_v[b]
            sc_ps = st_sc[b]
            attnT_tiles = []
            for hp in range(NHP):
                a_sc = attn.tile([P, 64], F32, tag="a_sc")
                nc.vector.tensor_copy(out=a_sc[0:64, :], in_=sc_ps[hp][0:64, 0:64])
                nc.vector.tensor_copy(out=a_sc[64:P, :], in_=sc_ps[hp][64:P, 64:P])
                mx = attn.tile([P, 1], F32, tag="mx")
                nc.vector.reduce_max(out=mx, in_=a_sc, axis=AX.X)
                nmx = attn.tile([P, 1], F32, tag="nmx")
                nc.vector.tensor_scalar_mul(nmx, mx, -0.125)
                a_e = attn.tile([P, 64], F32, tag="a_e")
                nc.scalar.activation(out=a_e, in_=a_sc, func=ACTF.Exp,
                                     bias=nmx, scale=0.125)
                sm = attn.tile([P, 1], F32, tag="sm")
                nc.vector.reduce_sum(out=sm, in_=a_e, axis=AX.X)
                rs = attn.tile([P, 1], F32, tag="rs")
                nc.vector.reciprocal(out=rs, in_=sm)
                a_mm = attn.tile([P, 64], BF16, tag="a_mm")
                nc.vector.tensor_scalar_mul(a_mm, a_e, rs)
                at = attnt.tile([P, P], BF16, tag="attnT", name=f"at_{b}_{hp}")
                nc.gpsimd.memset(at, 0.0)
                attnT_tiles.append((at, a_mm))

            for hp in range(NHP):
                at, a_mm = attnT_tiles[hp]
                pt = psmm.tile([P, 64], BF16, tag="psmm")
                nc.tensor.transpose(pt[0:64, :], a_mm[0:64, :], ident[0:64, 0:64])
                nc.tensor.transpose(pt[64:P, :], a_mm[64:P, :], ident[64:P, 64:P])
                nc.vector.tensor_copy(out=at[0:64, 0:64], in_=pt[0:64, :])
                nc.vector.tensor_copy(out=at[64:P, 64:P], in_=pt[64:P, :])

            ao = aopool.tile([P, KC, N], BF16, tag="ao", name=f"ao_{b}")
            for hp in range(NHP):
                at, _ = attnT_tiles[hp]
                for ns in range(NS):
                    ps = psmm.tile([P, 512], F32, tag="psmm")
                    nc.tensor.matmul(ps, lhsT=at,
                                     rhs=v_sb[:, hp, ns * 512:(ns + 1) * 512],
                                     start=True, stop=True)
                    nc.vector.tensor_copy(out=ao[:, hp, ns * 512:(ns + 1) * 512],
                                          in_=ps)
            st_ao[b] = ao

        def emit_By(b):
            """out projection + bn_stats."""
            ao = st_ao[b]
            y_lo = ypool.tile([P, 2, N], F32, tag="y", name=f"ylo_{b}")
            y_hi = ypool.tile([P, 2, N], F32, tag="y", name=f"yhi_{b}")
            st = stats.tile([P, KC, NS, 6], F32, tag="bnstats")
            st_y[b] = (y_lo, y_hi)
            st_stats[b] = st
            for m in range(KC):
                yt = y_lo if m < 2 else y_hi
                mi = m % 2
                for ns in range(NS):
                    ps = psmm.tile([P, 512], F32, tag="psmm")
                    for k in range(KC):
                        nc.tensor.matmul(
                            ps,
                            lhsT=wo_sb[:, k, m * P:(m + 1) * P],
                            rhs=ao[:, k, ns * 512:(ns + 1) * 512],
                            start=(k == 0), stop=(k == KC - 1))
                    # stats on pre-bias values (bias folded in below)
                    nc.vector.bn_stats(out=st[:, m, ns, :], in_=ps)
                    nc.scalar.add(out=yt[:, mi, ns * 512:(ns + 1) * 512],
                                  in_=ps, add=bias_sb[:, m:m + 1])

        st_scale = {}

        def emit_tail_stats(b):
            """global mean/var combine."""
            st = st_stats[b]
            mv = stats.tile([P, KC, 2], F32, tag="mv")
            for m in range(KC):
                nc.vector.bn_aggr(out=mv[:, m, :], in_=st[:, m])
            # S[p, stat, m]: 0 = mean+bias, 1 = var, 2 = (mean+bias)^2
            s_t = stats.tile([P, 3, KC], F32, tag="s_t")
            nc.vector.tensor_add(s_t[:, 0, :], mv[:, :, 0], bias_sb)
            nc.vector.tensor_copy(out=s_t[:, 1, :], in_=mv[:, :, 1])
            nc.vector.tensor_mul(s_t[:, 2, :], s_t[:, 0, :], s_t[:, 0, :])
            pstat = psmm.tile([1, 3, KC], F32, tag="psmm")
            nc.tensor.matmul(pstat, lhsT=ones_col, rhs=s_t,
                             start=True, stop=True)
            red = stats.tile([1, 3], F32, tag="red")
            nc.vector.reduce_sum(out=red, in_=pstat, axis=AX.X)
            e3 = stats.tile([1, 3], F32, tag="e3")
            nc.vector.tensor_scalar_mul(e3, red, 1.0 / C)
            m2 = stats.tile([1, 1], F32, tag="m2")
            nc.vector.tensor_mul(m2, e3[:, 0:1], e3[:, 0:1])
            var = stats.tile([1, 1], F32, tag="var")
            nc.vector.tensor_add(var, e3[:, 1:2], e3[:, 2:3])
            nc.vector.tensor_sub(var, var, m2)
            std = stats.tile([1, 1], F32, tag="std")
            nc.scalar.activation(out=std, in_=var, func=ACTF.Sqrt,
                                 bias=eps_sb, scale=1.0)
            rstd = stats.tile([1, 1], F32, tag="rstd")
            nc.vector.reciprocal(out=rstd, in_=std)
            sc2 = stats.tile([1, 2], F32, tag="sc2")
            nc.vector.tensor_copy(out=sc2[:, 0:1], in_=e3[:, 0:1])
            nc.vector.tensor_copy(out=sc2[:, 1:2], in_=rstd)
            bc_ps = psmm.tile([P, 2], F32, tag="psmm")
            nc.tensor.matmul(bc_ps, lhsT=ones_row, rhs=sc2,
                             start=True, stop=True)
            # s = gamma * rstd ; t = beta - mean_total * s
            s_ch = stats.tile([P, KC], F32, tag="s_ch")
            nc.vector.tensor_scalar_mul(s_ch, gamma_sb, bc_ps[:, 1:2])
            t_ch = stats.tile([P, KC], F32, tag="t_ch")
            nc.vector.tensor_scalar_mul(t_ch, s_ch, bc_ps[:, 0:1])
            nc.vector.tensor_sub(t_ch, beta_sb, t_ch)
            st_scale[b] = (s_ch, t_ch)

        def emit_tail_apply(b):
            """normalization apply + writeout."""
            y_lo, y_hi = st_y[b]
            s_ch, t_ch = st_scale[b]
            for m in range(KC):
                yt = y_lo if m < 2 else y_hi
                mi = m % 2
                for h in range(2):
                    sl = slice(h * (N // 2), (h + 1) * (N // 2))
                    if m % 2 == 0:
                        nc.vector.tensor_scalar(
                            out=yt[:, mi, sl], in0=yt[:, mi, sl],
                            scalar1=s_ch[:, m:m + 1], scalar2=t_ch[:, m:m + 1],
                            op0=ALU.mult, op1=ALU.add)
                    else:
                        nc.scalar.activation(
                            out=yt[:, mi, sl], in_=yt[:, mi, sl],
                            func=ACTF.Identity,
                            bias=t_ch[:, m:m + 1], scale=s_ch[:, m:m + 1])
                    nc.sync.dma_start(out=out_d[b, m * P:(m + 1) * P, sl],
                                      in_=yt[:, mi, sl])

        emit_A(0)
        emit_Bhead(0)
        emit_By(0)
        emit_A(1)
        emit_tail_stats(0)
        emit_Bhead(1)
        emit_tail_apply(0)
        emit_By(1)
        emit_tail_stats(1)
        emit_tail_apply(1)

    nc.finalize()
    return nc


_NC_CACHE = {}


def _get_nc():
    if "nc" not in _NC_CACHE:
        _NC_CACHE["nc"] = build_nc()
    return _NC_CACHE["nc"]


def _prep_w(w):
    # [C_in, C_out] -> [128, KC, C_out] with c_in = k*128 + p, contiguous
    return np.ascontiguousarray(w.reshape(KC, P, C).transpose(1, 0, 2))


def _prep_vec(v):
    # [C] -> [128, KC] with c = k*128 + p
    return np.ascontiguousarray(v.reshape(KC, P).T)


def _prep_x(x):
    # [B, C, N] -> [B, NB, P, KC*512]: block j, partition p, (k, n) contiguous
    nb = np.asarray(x).shape[0]
    xr = np.asarray(x, dtype=np.float32).reshape(nb, KC, P, NB, 512)
    return np.ascontiguousarray(xr.transpose(0, 3, 2, 1, 4)).reshape(
        nb, NB, P, KC * 512)


def _prep_x_local(x):
    return _prep_x(x)


def _make_in_maps(x, w_qkv, w_out, b_out, gamma, beta):
    xr = _prep_x(x)
    w_qkv = np.asarray(w_qkv, dtype=np.float32)
    wq = _prep_w(np.ascontiguousarray(w_qkv[0:C].T))
    wk = _prep_w(np.ascontiguousarray(w_qkv[C:2 * C].T))
    wv = _prep_w(np.ascontiguousarray(w_qkv[2 * C:3 * C].T))
    wo = _prep_w(np.ascontiguousarray(np.asarray(w_out, dtype=np.float32).T))
    bvec = _prep_vec(np.asarray(b_out, dtype=np.float32))
    gam = _prep_vec(np.asarray(gamma, dtype=np.float32))
    bet = _prep_vec(np.asarray(beta, dtype=np.float32))
    return [
        dict(x=np.ascontiguousarray(xr[c * PB:(c + 1) * PB]),
             wq=wq, wk=wk, wv=wv, wo=wo,
             bvec=bvec, gamma=gam, beta=bet)
        for c in range(NCORES)
    ]


def _run(inputs, trace=False, trace_kwargs=None):
    nc = _get_nc()
    in_maps = _make_in_maps(**inputs)
    res = run_bass_kernel_spmd(nc, in_maps, core_ids=list(range(NCORES)),
                               trace=trace, **(trace_kwargs or {}))
    out = np.concatenate([res.results[c]["out"] for c in range(NCORES)], axis=0)
    return out.reshape(B, C, HW_SIDE, HW_SIDE), res


def kernel(x, w_qkv, w_out, b_out, gamma, beta):
    out, _ = _run(dict(x=x, w_qkv=w_qkv, w_out=w_out, b_out=b_out,
                       gamma=gamma, beta=beta))
    return out


# revision 11
# speedup vs baseline: 1.1244x; 1.0648x over previous
"""Trainium2 Bass kernel for nn_MultiHeadAttention_63814624084186.

Reference computation (per batch sample b, fully independent across b):
  x: [512, 4096]  (C channels x N=64*64 pixels)
  qkv = w_qkv @ x            -> q,k,v each [512, 4096] (8 heads x 64 dims)
  scores = (q_h @ k_h^T)/8   -> [64, 64] per head   (channel-attention)
  attn = softmax(scores, -1)
  out_h = attn_h @ v_h       -> [64, 4096]
  y = w_out @ out + b_out    -> [512, 4096]
  y = groupnorm(y over all C,N) * gamma + beta

Sharding: pure data-parallel over batch: 16 samples / 8 cores = 2 per core.

Design notes:
  - q/k/v GEMMs run in float32r (tf32-class precision at bf16-like speed
    for N=512).  x and w_q/w_k/w_v are DMA'd straight into float32r tiles
    (PE rounds on read; verified on HW).
  - phase 1+2 are n-blocked (8 blocks of 512 pixels): per block we DMA an
    x block (one DMA per channel chunk), compute qT/kT blocks ([N,512]
    layout via GEMM "transpose": lhsT = x block), accumulate scores into
    4 persistent PSUM banks, and compute v for the block.
  - scores/attn@v/out-proj run in bf16 (error contribution ~3e-3).
  - GroupNorm: bn_stats per PSUM tile (bias folded into the cross-
    partition combine), cross-partition reduce via ones-matmul.
  - The two batches are emitted interleaved:
    A(0) B(0) A(1) tail(0) B(1) tail(1), where A = blocked qkv+scores,
    B = softmax/attn@v/out-proj/bn_stats, tail = stat combine+apply+store.
    This hides batch 0's epilogue fully under batch 1's compute and keeps
    the PE queue free of stat matmuls between batches.
  - Weights arrive host-prearranged as [128, KC, C] so weight DMAs are
    contiguous per partition (few descriptors, fast issue).
"""

import numpy as np
from contextlib import ExitStack

import concourse.bass as bass
import concourse.tile as tile
from concourse import bacc, mybir
from concourse.bass_utils import run_bass_kernel_spmd
from concourse.masks import make_identity

F32 = mybir.dt.float32
F32R = mybir.dt.float32r
F16 = mybir.dt.float16
BF16 = mybir.dt.bfloat16
AX = mybir.AxisListType
ALU = mybir.AluOpType
ACTF = mybir.ActivationFunctionType

B = 16          # global batch
C = 512         # channels
N = 4096        # pixels (64*64)
HW_SIDE = 64
NCORES = 8
PB = B // NCORES  # batches per core
P = 128
KC = C // P     # 4 channel chunks
NB = 8          # n blocks of 512
NBI = 4         # 128-chunks per n block
NS = N // 512   # 8 pixel chunks of 512
NHP = 4         # head pairs
EPS = 1e-5


def build_nc():
    nc = bacc.Bacc("TRN2", target_bir_lowering=False, debug=False,
                   num_devices=NCORES)

    x_d = nc.declare_dram_parameter("x", [PB, NB, P, KC * 512], F16, isOutput=False)
    wq_d = nc.declare_dram_parameter("wq", [P, KC, C], F16, isOutput=False)
    wk_d = nc.declare_dram_parameter("wk", [P, KC, C], F16, isOutput=False)
    wv_d = nc.declare_dram_parameter("wv", [P, KC, C], F16, isOutput=False)
    wo_d = nc.declare_dram_parameter("wo", [P, KC, C], F16, isOutput=False)
    bias_d = nc.declare_dram_parameter("bvec", [P, KC], F32, isOutput=False)
    gamma_d = nc.declare_dram_parameter("gamma", [P, KC], F32, isOutput=False)
    beta_d = nc.declare_dram_parameter("beta", [P, KC], F32, isOutput=False)
    out_d = nc.declare_dram_parameter("out", [PB, C, N], F32, isOutput=True)

    with tile.TileContext(nc) as tc, ExitStack() as ctx:
        consts = ctx.enter_context(tc.tile_pool(name="consts", bufs=1))
        xpool = ctx.enter_context(tc.tile_pool(name="xpool", bufs=3))
        qkpool = ctx.enter_context(tc.tile_pool(name="qkpool", bufs=4))
        vpool = ctx.enter_context(tc.tile_pool(name="vpool", bufs=1))
        aopool = ctx.enter_context(tc.tile_pool(name="aopool", bufs=1))
        ypool = ctx.enter_context(tc.tile_pool(name="ypool", bufs=2))
        attn = ctx.enter_context(tc.tile_pool(name="attn", bufs=8))
        attnt = ctx.enter_context(tc.tile_pool(name="attnt", bufs=4))
        stats = ctx.enter_context(tc.tile_pool(name="stats", bufs=2))
        psmm = ctx.enter_context(tc.tile_pool(name="psmm", bufs=4, space="PSUM"))
        pssc = ctx.enter_context(tc.tile_pool(name="pssc", bufs=4, space="PSUM"))

        # ---- prefetch first x blocks before weights (lead-in) ----
        # ---- interleave weight and x-block loads for minimal lead-in ----
        def load_w(dram):
            t = consts.tile([P, KC, C], F16, tag=f"w_{dram.name}")
            nc.sync.dma_start(out=t, in_=dram[:, :, :])
            return t

        prefetched_x = {}

        def prefetch_x(j):
            xt = xpool.tile([P, KC, 512], F16, tag="xblk", name=f"x_0_{j}")
            nc.sync.dma_start(
                out=xt, in_=x_d[0, j].rearrange("p (k n) -> p k n", k=KC))
            prefetched_x[j] = xt

        wq_sb = load_w(wq_d)
        prefetch_x(0)
        wk_sb = load_w(wk_d)
        prefetch_x(1)
        wv_sb = load_w(wv_d)
        prefetch_x(2)
        wo_sb = load_w(wo_d)

        bias_sb = consts.tile([P, KC], F32, tag="bias")
        nc.gpsimd.dma_start(out=bias_sb, in_=bias_d[:, :])
        gamma_sb = consts.tile([P, KC], F32, tag="gamma")
        nc.gpsimd.dma_start(out=gamma_sb, in_=gamma_d[:, :])
        beta_sb = consts.tile([P, KC], F32, tag="beta")
        nc.gpsimd.dma_start(out=beta_sb, in_=beta_d[:, :])

        ident = consts.tile([P, P], F16, tag="ident")
        make_identity(nc, ident)

        eps_sb = consts.tile([1, 1], F32, tag="eps")
        nc.vector.memset(eps_sb, EPS)
        ones_col = consts.tile([P, 1], F32, tag="ones_col")
        nc.vector.memset(ones_col, 1.0)
        ones_row = consts.tile([1, P], F32, tag="ones_row")
        nc.vector.memset(ones_row, 1.0)

        # per-batch state carried between emission stages
        st_v = {}
        st_sc = {}
        st_y = {}
        st_stats = {}

        def emit_A(b):
            """n-blocked qT/kT GEMMs, score accumulation, v GEMM."""
            v_sb = vpool.tile([P, NHP, N], F16, tag="v", name=f"v_{b}")
            sc_ps = [pssc.tile([P, P], F32, tag="pssc", name=f"sc_{b}_{hp}")
                     for hp in range(NHP)]
            st_v[b] = v_sb
            st_sc[b] = sc_ps
            for j in range(NB):
                if b == 0 and j in prefetched_x:
                    x_blk = prefetched_x[j]
                else:
                    x_blk = xpool.tile([P, KC, 512], F16, tag="xblk",
                                       name=f"x_{b}_{j}")
                    nc.sync.dma_start(
                        out=x_blk,
                        in_=x_d[b, j].rearrange("p (k n) -> p k n", k=KC))

                qT_blk = qkpool.tile([P, NBI, C], F16, tag="qk",
                                     name=f"qT_{b}_{j}")
                kT_blk = qkpool.tile([P, NBI, C], F16, tag="qk",
                                     name=f"kT_{b}_{j}")
                for dst, w in ((qT_blk, wq_sb), (kT_blk, wk_sb)):
                    for i in range(NBI):
                        ps = psmm.tile([P, C], F32, tag="psmm")
                        for k in range(KC):
                            nc.tensor.matmul(
                                ps,
                                lhsT=x_blk[:, k, i * P:(i + 1) * P],
                                rhs=w[:, k, :],
                                start=(k == 0), stop=(k == KC - 1))
                        nc.scalar.copy(out=dst[:, i, :], in_=ps)

                for hp in range(NHP):
                    cl = slice(hp * P, (hp + 1) * P)
                    for i in range(NBI):
                        nc.tensor.matmul(
                            sc_ps[hp],
                            lhsT=qT_blk[:, i, cl],
                            rhs=kT_blk[:, i, cl],
                            start=(j == 0 and i == 0),
                            stop=(j == NB - 1 and i == NBI - 1),
                            skip_group_check=True)

                for hp in range(NHP):
                    cl = slice(hp * P, (hp + 1) * P)
                    ps = psmm.tile([P, 512], F32, tag="psmm")
                    for k in range(KC):
                        nc.tensor.matmul(
                            ps,
                            lhsT=wv_sb[:, k, cl],
                            rhs=x_blk[:, k, :],
                            start=(k == 0), stop=(k == KC - 1))
                    nc.vector.tensor_copy(
                        out=v_sb[:, hp, j * 512:(j + 1) * 512], in_=ps)

        st_ao = {}

        def emit_Bhead(b):
            """softmax, attn transposes, attn@v."""
            v_sb = st_v[b]
            sc_ps = st_sc[b]
            attnT_tiles = []
            for hp in range(NHP):
                a_sc = attn.tile([P, 64], F32, tag="a_sc")
                nc.vector.tensor_copy(out=a_sc[0:64, :], in_=sc_ps[hp][0:64, 0:64])
                nc.vector.tensor_copy(out=a_sc[64:P, :], in_=sc_ps[hp][64:P, 64:P])
                mx = attn.tile([P, 1], F32, tag="mx")
                nc.vector.reduce_max(out=mx, in_=a_sc, axis=AX.X)
                nmx = attn.tile([P, 1], F32, tag="nmx")
                nc.vector.tensor_scalar_mul(nmx, mx, -0.125)
                a_e = attn.tile([P, 64], F32, tag="a_e")
                nc.scalar.activation(out=a_e, in_=a_sc, func=ACTF.Exp,
                                     bias=nmx, scale=0.125)
                sm = attn.tile([P, 1], F32, tag="sm")
                nc.vector.reduce_sum(out=sm, in_=a_e, axis=AX.X)
                rs = attn.tile([P, 1], F32, tag="rs")
                nc.vector.reciprocal(out=rs, in_=sm)
                a_mm = attn.tile([P, 64], F16, tag="a_mm")
                nc.vector.tensor_scalar_mul(a_mm, a_e, rs)
                at = attnt.tile([P, P], F16, tag="attnT", name=f"at_{b}_{hp}")
                nc.gpsimd.memset(at, 0.0)
                attnT_tiles.append((at, a_mm))

            ao = aopool.tile([P, KC, N], F16, tag="ao", name=f"ao_{b}")
            for hp in range(NHP):
                at, a_mm = attnT_tiles[hp]
                pt = psmm.tile([P, 64], F16, tag="psmm")
                nc.tensor.transpose(pt[0:64, :], a_mm[0:64, :], ident[0:64, 0:64])
                nc.tensor.transpose(pt[64:P, :], a_mm[64:P, :], ident[64:P, 64:P])
                nc.vector.tensor_copy(out=at[0:64, 0:64], in_=pt[0:64, :])
                nc.vector.tensor_copy(out=at[64:P, 64:P], in_=pt[64:P, :])
                for ns in range(NS):
                    ps = psmm.tile([P, 512], F32, tag="psmm")
                    nc.tensor.matmul(ps, lhsT=at,
                                     rhs=v_sb[:, hp, ns * 512:(ns + 1) * 512],
                                     start=True, stop=True)
                    nc.vector.tensor_copy(out=ao[:, hp, ns * 512:(ns + 1) * 512],
                                          in_=ps)
            st_ao[b] = ao

        def emit_By(b):
            """out projection + bn_stats."""
            ao = st_ao[b]
            y_lo = ypool.tile([P, 2, N], F32, tag="y", name=f"ylo_{b}")
            y_hi = ypool.tile([P, 2, N], F32, tag="y", name=f"yhi_{b}")
            st = stats.tile([P, KC, NS, 6], F32, tag="bnstats")
            st_y[b] = (y_lo, y_hi)
            st_stats[b] = st
            for m in range(KC):
                yt = y_lo if m < 2 else y_hi
                mi = m % 2
                for ns in range(NS):
                    ps = psmm.tile([P, 512], F32, tag="psmm")
                    for k in range(KC):
                        nc.tensor.matmul(
                            ps,
                            lhsT=wo_sb[:, k, m * P:(m + 1) * P],
                            rhs=ao[:, k, ns * 512:(ns + 1) * 512],
                            start=(k == 0), stop=(k == KC - 1))
                    # stats on pre-bias values (bias folded in below)
                    nc.vector.bn_stats(out=st[:, m, ns, :], in_=ps)
                    nc.scalar.add(out=yt[:, mi, ns * 512:(ns + 1) * 512],
                                  in_=ps, add=bias_sb[:, m:m + 1])

        st_scale = {}

        def emit_tail_stats(b):
            """global mean/var combine."""
            st = st_stats[b]
            mv = stats.tile([P, KC, 2], F32, tag="mv")
            for m in range(KC):
                nc.vector.bn_aggr(out=mv[:, m, :], in_=st[:, m])
            # S[p, stat, m]: 0 = mean+bias, 1 = var, 2 = (mean+bias)^2
            s_t = stats.tile([P, 3, KC], F32, tag="s_t")
            nc.vector.tensor_add(s_t[:, 0, :], mv[:, :, 0], bias_sb)
            nc.vector.tensor_copy(out=s_t[:, 1, :], in_=mv[:, :, 1])
            nc.vector.tensor_mul(s_t[:, 2, :], s_t[:, 0, :], s_t[:, 0, :])
            pstat = psmm.tile([1, 3, KC], F32, tag="psmm")
            nc.tensor.matmul(pstat, lhsT=ones_col, rhs=s_t,
                             start=True, stop=True)
            red = stats.tile([1, 3], F32, tag="red")
            nc.vector.reduce_sum(out=red, in_=pstat, axis=AX.X)
            e3 = stats.tile([1, 3], F32, tag="e3")
            nc.vector.tensor_scalar_mul(e3, red, 1.0 / C)
            m2 = stats.tile([1, 1], F32, tag="m2")
            nc.vector.tensor_mul(m2, e3[:, 0:1], e3[:, 0:1])
            var = stats.tile([1, 1], F32, tag="var")
            nc.vector.tensor_add(var, e3[:, 1:2], e3[:, 2:3])
            nc.vector.tensor_sub(var, var, m2)
            std = stats.tile([1, 1], F32, tag="std")
            nc.scalar.activation(out=std, in_=var, func=ACTF.Sqrt,
                                 bias=eps_sb, scale=1.0)
            rstd = stats.tile([1, 1], F32, tag="rstd")
            nc.vector.reciprocal(out=rstd, in_=std)
            sc2 = stats.tile([1, 2], F32, tag="sc2")
            nc.vector.tensor_copy(out=sc2[:, 0:1], in_=e3[:, 0:1])
            nc.vector.tensor_copy(out=sc2[:, 1:2], in_=rstd)
            bc_ps = psmm.tile([P, 2], F32, tag="psmm")
            nc.tensor.matmul(bc_ps, lhsT=ones_row, rhs=sc2,
                             start=True, stop=True)
            # s = gamma * rstd ; t = beta - mean_total * s
            s_ch = stats.tile([P, KC], F32, tag="s_ch")
            nc.vector.tensor_scalar_mul(s_ch, gamma_sb, bc_ps[:, 1:2])
            t_ch = stats.tile([P, KC], F32, tag="t_ch")
            nc.vector.tensor_scalar_mul(t_ch, s_ch, bc_ps[:, 0:1])
            nc.vector.tensor_sub(t_ch, beta_sb, t_ch)
            st_scale[b] = (s_ch, t_ch)

        def emit_tail_apply(b):
            """normalization apply + writeout."""
            y_lo, y_hi = st_y[b]
            s_ch, t_ch = st_scale[b]
            for m in range(KC):
                yt = y_lo if m < 2 else y_hi
                mi = m % 2
                for h in range(2):
                    sl = slice(h * (N // 2), (h + 1) * (N // 2))
                    if m % 2 == 0:
                        nc.vector.tensor_scalar(
                            out=yt[:, mi, sl], in0=yt[:, mi, sl],
                            scalar1=s_ch[:, m:m + 1], scalar2=t_ch[:, m:m + 1],
                            op0=ALU.mult, op1=ALU.add)
                    else:
                        nc.scalar.activation(
                            out=yt[:, mi, sl], in_=yt[:, mi, sl],
                            func=ACTF.Identity,
                            bias=t_ch[:, m:m + 1], scale=s_ch[:, m:m + 1])
                    nc.sync.dma_start(out=out_d[b, m * P:(m + 1) * P, sl],
                                      in_=yt[:, mi, sl])

        emit_A(0)
        emit_Bhead(0)
        emit_By(0)
        emit_A(1)
        emit_tail_stats(0)
        emit_Bhead(1)
        emit_tail_apply(0)
        emit_By(1)
        emit_tail_stats(1)
        emit_tail_apply(1)

    nc.finalize()
    return nc


_NC_CACHE = {}


def _get_nc():
    if "nc" not in _NC_CACHE:
        _NC_CACHE["nc"] = build_nc()
    return _NC_CACHE["nc"]


def _prep_w(w):
    # [C_in, C_out] -> [128, KC, C_out] fp16 with c_in = k*128 + p
    return np.ascontiguousarray(
        w.reshape(KC, P, C).transpose(1, 0, 2).astype(np.float16))


def _prep_vec(v):
    # [C] -> [128, KC] with c = k*128 + p
    return np.ascontiguousarray(v.reshape(KC, P).T)


def _prep_x(x):
    # [B, C, N] -> [B, NB, P, KC*512] fp16: block j, partition p, (k, n)
    nb = np.asarray(x).shape[0]
    xr = np.asarray(x, dtype=np.float32).reshape(nb, KC, P, NB, 512)
    return np.ascontiguousarray(
        xr.transpose(0, 3, 2, 1, 4).astype(np.float16)).reshape(
        nb, NB, P, KC * 512)


def _prep_x_local(x):
    return _prep_x(x)


def _make_in_maps(x, w_qkv, w_out, b_out, gamma, beta):
    xr = _prep_x(x)
    w_qkv = np.asarray(w_qkv, dtype=np.float32)
    wq = _prep_w(np.ascontiguousarray(w_qkv[0:C].T))
    wk = _prep_w(np.ascontiguousarray(w_qkv[C:2 * C].T))
    wv = _prep_w(np.ascontiguousarray(w_qkv[2 * C:3 * C].T))
    wo = _prep_w(np.ascontiguousarray(np.asarray(w_out, dtype=np.float32).T))
    bvec = _prep_vec(np.asarray(b_out, dtype=np.float32))
    gam = _prep_vec(np.asarray(gamma, dtype=np.float32))
    bet = _prep_vec(np.asarray(beta, dtype=np.float32))
    return [
        dict(x=np.ascontiguousarray(xr[c * PB:(c + 1) * PB]),
             wq=wq, wk=wk, wv=wv, wo=wo,
             bvec=bvec, gamma=gam, beta=bet)
        for c in range(NCORES)
    ]


def _run(inputs, trace=False, trace_kwargs=None):
    nc = _get_nc()
    in_maps = _make_in_maps(**inputs)
    res = run_bass_kernel_spmd(nc, in_maps, core_ids=list(range(NCORES)),
                               trace=trace, **(trace_kwargs or {}))
    out = np.concatenate([res.results[c]["out"] for c in range(NCORES)], axis=0)
    return out.reshape(B, C, HW_SIDE, HW_SIDE), res


def kernel(x, w_qkv, w_out, b_out, gamma, beta):
    out, _ = _run(dict(x=x, w_qkv=w_qkv, w_out=w_out, b_out=b_out,
                       gamma=gamma, beta=beta))
    return out


# revision 17
# speedup vs baseline: 1.1281x; 1.0033x over previous
"""Trainium2 Bass kernel for nn_MultiHeadAttention_63814624084186.

Reference computation (per batch sample b, fully independent across b):
  x: [512, 4096]  (C channels x N=64*64 pixels)
  qkv = w_qkv @ x            -> q,k,v each [512, 4096] (8 heads x 64 dims)
  scores = (q_h @ k_h^T)/8   -> [64, 64] per head   (channel-attention)
  attn = softmax(scores, -1)
  out_h = attn_h @ v_h       -> [64, 4096]
  y = w_out @ out + b_out    -> [512, 4096]
  y = groupnorm(y over all C,N) * gamma + beta

Sharding: pure data-parallel over batch: 16 samples / 8 cores = 2 per core.

Design notes:
  - q/k/v GEMMs run in float32r (tf32-class precision at bf16-like speed
    for N=512).  x and w_q/w_k/w_v are DMA'd straight into float32r tiles
    (PE rounds on read; verified on HW).
  - phase 1+2 are n-blocked (8 blocks of 512 pixels): per block we DMA an
    x block (one DMA per channel chunk), compute qT/kT blocks ([N,512]
    layout via GEMM "transpose": lhsT = x block), accumulate scores into
    4 persistent PSUM banks, and compute v for the block.
  - scores/attn@v/out-proj run in bf16 (error contribution ~3e-3).
  - GroupNorm: bn_stats per PSUM tile (bias folded into the cross-
    partition combine), cross-partition reduce via ones-matmul.
  - The two batches are emitted interleaved:
    A(0) B(0) A(1) tail(0) B(1) tail(1), where A = blocked qkv+scores,
    B = softmax/attn@v/out-proj/bn_stats, tail = stat combine+apply+store.
    This hides batch 0's epilogue fully under batch 1's compute and keeps
    the PE queue free of stat matmuls between batches.
  - Weights arrive host-prearranged as [128, KC, C] so weight DMAs are
    contiguous per partition (few descriptors, fast issue).
"""

import numpy as np
from contextlib import ExitStack

import concourse.bass as bass
import concourse.tile as tile
from concourse import bacc, mybir
from concourse.bass_utils import run_bass_kernel_spmd
from concourse.masks import make_identity

F32 = mybir.dt.float32
F32R = mybir.dt.float32r
F16 = mybir.dt.float16
BF16 = mybir.dt.bfloat16
AX = mybir.AxisListType
ALU = mybir.AluOpType
ACTF = mybir.ActivationFunctionType

B = 16          # global batch
C = 512         # channels
N = 4096        # pixels (64*64)
HW_SIDE = 64
NCORES = 8
PB = B // NCORES  # batches per core
P = 128
KC = C // P     # 4 channel chunks
NB = 8          # n blocks of 512
NBI = 4         # 128-chunks per n block
NS = N // 512   # 8 pixel chunks of 512
NHP = 4         # head pairs
EPS = 1e-5


def build_nc():
    nc = bacc.Bacc("TRN2", target_bir_lowering=False, debug=False,
                   num_devices=NCORES)

    x_d = nc.declare_dram_parameter("x", [PB, NB, P, KC * 512], F16, isOutput=False)
    wq_d = nc.declare_dram_parameter("wq", [P, KC, C], F16, isOutput=False)
    wk_d = nc.declare_dram_parameter("wk", [P, KC, C], F16, isOutput=False)
    wv_d = nc.declare_dram_parameter("wv", [P, KC, C], F16, isOutput=False)
    wo_d = nc.declare_dram_parameter("wo", [P, KC, C], F16, isOutput=False)
    bias_d = nc.declare_dram_parameter("bvec", [P, KC], F32, isOutput=False)
    gamma_d = nc.declare_dram_parameter("gamma", [P, KC], F32, isOutput=False)
    beta_d = nc.declare_dram_parameter("beta", [P, KC], F32, isOutput=False)
    out_d = nc.declare_dram_parameter("out", [PB, C, N], F32, isOutput=True)

    with tile.TileContext(nc) as tc, ExitStack() as ctx:
        consts = ctx.enter_context(tc.tile_pool(name="consts", bufs=1))
        xpool = ctx.enter_context(tc.tile_pool(name="xpool", bufs=3))
        qkpool = ctx.enter_context(tc.tile_pool(name="qkpool", bufs=3))
        vpool = ctx.enter_context(tc.tile_pool(name="vpool", bufs=2))
        aopool = ctx.enter_context(tc.tile_pool(name="aopool", bufs=1))
        ypool = ctx.enter_context(tc.tile_pool(name="ypool", bufs=2))
        attn = ctx.enter_context(tc.tile_pool(name="attn", bufs=6))
        attnt = ctx.enter_context(tc.tile_pool(name="attnt", bufs=4))
        stats = ctx.enter_context(tc.tile_pool(name="stats", bufs=1))
        psmm = ctx.enter_context(tc.tile_pool(name="psmm", bufs=4, space="PSUM"))
        pssc = ctx.enter_context(tc.tile_pool(name="pssc", bufs=4, space="PSUM"))

        # ---- prefetch first x blocks before weights (lead-in) ----
        # ---- interleave weight and x-block loads for minimal lead-in ----
        def load_w(dram):
            t = consts.tile([P, KC, C], F16, tag=f"w_{dram.name}")
            nc.sync.dma_start(out=t, in_=dram[:, :, :])
            return t

        prefetched_x = {}

        def prefetch_x(j):
            xt = xpool.tile([P, KC, 512], F16, tag="xblk", name=f"x_0_{j}")
            nc.sync.dma_start(
                out=xt, in_=x_d[0, j].rearrange("p (k n) -> p k n", k=KC))
            prefetched_x[j] = xt

        wq_sb = load_w(wq_d)
        prefetch_x(0)
        wk_sb = load_w(wk_d)
        prefetch_x(1)
        wv_sb = load_w(wv_d)
        prefetch_x(2)
        wo_sb = load_w(wo_d)

        bias_sb = consts.tile([P, KC], F32, tag="bias")
        nc.gpsimd.dma_start(out=bias_sb, in_=bias_d[:, :])
        gamma_sb = consts.tile([P, KC], F32, tag="gamma")
        nc.gpsimd.dma_start(out=gamma_sb, in_=gamma_d[:, :])
        beta_sb = consts.tile([P, KC], F32, tag="beta")
        nc.gpsimd.dma_start(out=beta_sb, in_=beta_d[:, :])

        ident = consts.tile([P, P], F16, tag="ident")
        make_identity(nc, ident)

        eps_sb = consts.tile([1, 1], F32, tag="eps")
        nc.vector.memset(eps_sb, EPS)
        ones_col = consts.tile([P, 1], F32, tag="ones_col")
        nc.vector.memset(ones_col, 1.0)
        ones_row = consts.tile([1, P], F32, tag="ones_row")
        nc.vector.memset(ones_row, 1.0)

        # per-batch state carried between emission stages
        st_v = {}
        st_sc = {}
        st_y = {}
        st_stats = {}

        def emit_A_setup(b):
            v_sb = vpool.tile([P, NHP, N], F16, tag="v", name=f"v_{b}")
            sc_ps = [pssc.tile([P, P], F32, tag="pssc", name=f"sc_{b}_{hp}")
                     for hp in range(NHP)]
            st_v[b] = v_sb
            st_sc[b] = sc_ps

        def emit_A_blocks(b, blocks):
            """n-blocked qT/kT GEMMs, score accumulation, v GEMM."""
            v_sb = st_v[b]
            sc_ps = st_sc[b]
            for j in blocks:
                if b == 0 and j in prefetched_x:
                    x_blk = prefetched_x[j]
                else:
                    x_blk = xpool.tile([P, KC, 512], F16, tag="xblk",
                                       name=f"x_{b}_{j}")
                    nc.sync.dma_start(
                        out=x_blk,
                        in_=x_d[b, j].rearrange("p (k n) -> p k n", k=KC))

                qT_blk = qkpool.tile([P, NBI, C], F16, tag="qk",
                                     name=f"qT_{b}_{j}")
                kT_blk = qkpool.tile([P, NBI, C], F16, tag="qk",
                                     name=f"kT_{b}_{j}")
                for dst, w in ((qT_blk, wq_sb), (kT_blk, wk_sb)):
                    for i in range(NBI):
                        ps = psmm.tile([P, C], F32, tag="psmm")
                        for k in range(KC):
                            nc.tensor.matmul(
                                ps,
                                lhsT=x_blk[:, k, i * P:(i + 1) * P],
                                rhs=w[:, k, :],
                                start=(k == 0), stop=(k == KC - 1))
                        nc.scalar.copy(out=dst[:, i, :], in_=ps)

                for hp in range(NHP):
                    cl = slice(hp * P, (hp + 1) * P)
                    for i in range(NBI):
                        nc.tensor.matmul(
                            sc_ps[hp],
                            lhsT=qT_blk[:, i, cl],
                            rhs=kT_blk[:, i, cl],
                            start=(j == 0 and i == 0),
                            stop=(j == NB - 1 and i == NBI - 1),
                            skip_group_check=True)

                for hp in range(NHP):
                    cl = slice(hp * P, (hp + 1) * P)
                    ps = psmm.tile([P, 512], F32, tag="psmm")
                    for k in range(KC):
                        nc.tensor.matmul(
                            ps,
                            lhsT=wv_sb[:, k, cl],
                            rhs=x_blk[:, k, :],
                            start=(k == 0), stop=(k == KC - 1))
                    nc.vector.tensor_copy(
                        out=v_sb[:, hp, j * 512:(j + 1) * 512], in_=ps)

        st_ao = {}
        st_at = {}

        def emit_softmax(b):
            """softmax on the accumulated score blocks."""
            sc_ps = st_sc[b]
            attnT_tiles = []
            for hp in range(NHP):
                a_sc = attn.tile([P, 64], F32, tag="a_sc")
                nc.vector.tensor_copy(out=a_sc[0:64, :], in_=sc_ps[hp][0:64, 0:64])
                nc.vector.tensor_copy(out=a_sc[64:P, :], in_=sc_ps[hp][64:P, 64:P])
                mx = attn.tile([P, 1], F32, tag="mx")
                nc.vector.reduce_max(out=mx, in_=a_sc, axis=AX.X)
                nmx = attn.tile([P, 1], F32, tag="nmx")
                nc.vector.tensor_scalar_mul(nmx, mx, -0.125)
                a_e = attn.tile([P, 64], F32, tag="a_e")
                nc.scalar.activation(out=a_e, in_=a_sc, func=ACTF.Exp,
                                     bias=nmx, scale=0.125)
                sm = attn.tile([P, 1], F32, tag="sm")
                nc.vector.reduce_sum(out=sm, in_=a_e, axis=AX.X)
                rs = attn.tile([P, 1], F32, tag="rs")
                nc.vector.reciprocal(out=rs, in_=sm)
                a_mm = attn.tile([P, 64], F16, tag="a_mm")
                nc.vector.tensor_scalar_mul(a_mm, a_e, rs)
                at = attnt.tile([P, P], F16, tag="attnT", name=f"at_{b}_{hp}")
                nc.gpsimd.memset(at, 0.0)
                attnT_tiles.append((at, a_mm))
            st_at[b] = attnT_tiles

        def emit_attnv(b):
            """attn transposes + attn @ v."""
            v_sb = st_v[b]
            attnT_tiles = st_at[b]
            ao = aopool.tile([P, KC, N], F16, tag="ao", name=f"ao_{b}")
            for hp in range(NHP):
                at, a_mm = attnT_tiles[hp]
                pt = psmm.tile([P, 64], F16, tag="psmm")
                nc.tensor.transpose(pt[0:64, :], a_mm[0:64, :], ident[0:64, 0:64])
                nc.tensor.transpose(pt[64:P, :], a_mm[64:P, :], ident[64:P, 64:P])
                nc.vector.tensor_copy(out=at[0:64, 0:64], in_=pt[0:64, :])
                nc.vector.tensor_copy(out=at[64:P, 64:P], in_=pt[64:P, :])
                for ns in range(NS):
                    ps = psmm.tile([P, 512], F32, tag="psmm")
                    nc.tensor.matmul(ps, lhsT=at,
                                     rhs=v_sb[:, hp, ns * 512:(ns + 1) * 512],
                                     start=True, stop=True)
                    nc.vector.tensor_copy(out=ao[:, hp, ns * 512:(ns + 1) * 512],
                                          in_=ps)
            st_ao[b] = ao

        def emit_By(b):
            """out projection + bn_stats."""
            ao = st_ao[b]
            y_lo = ypool.tile([P, 2, N], F32, tag="y", name=f"ylo_{b}")
            y_hi = ypool.tile([P, 2, N], F32, tag="y", name=f"yhi_{b}")
            st = stats.tile([P, KC, NS, 6], F32, tag="bnstats")
            st_y[b] = (y_lo, y_hi)
            st_stats[b] = st
            for m in range(KC):
                yt = y_lo if m < 2 else y_hi
                mi = m % 2
                for ns in range(NS):
                    ps = psmm.tile([P, 512], F32, tag="psmm")
                    for k in range(KC):
                        nc.tensor.matmul(
                            ps,
                            lhsT=wo_sb[:, k, m * P:(m + 1) * P],
                            rhs=ao[:, k, ns * 512:(ns + 1) * 512],
                            start=(k == 0), stop=(k == KC - 1))
                    # stats on pre-bias values (bias folded in below)
                    nc.vector.bn_stats(out=st[:, m, ns, :], in_=ps)
                    nc.scalar.add(out=yt[:, mi, ns * 512:(ns + 1) * 512],
                                  in_=ps, add=bias_sb[:, m:m + 1])

        st_scale = {}

        def emit_tail_stats(b):
            """global mean/var combine."""
            st = st_stats[b]
            mv = stats.tile([P, KC, 2], F32, tag="mv")
            for m in range(KC):
                nc.vector.bn_aggr(out=mv[:, m, :], in_=st[:, m])
            # S[p, stat, m]: 0 = mean+bias, 1 = var, 2 = (mean+bias)^2
            s_t = stats.tile([P, 3, KC], F32, tag="s_t")
            nc.vector.tensor_add(s_t[:, 0, :], mv[:, :, 0], bias_sb)
            nc.vector.tensor_copy(out=s_t[:, 1, :], in_=mv[:, :, 1])
            nc.vector.tensor_mul(s_t[:, 2, :], s_t[:, 0, :], s_t[:, 0, :])
            pstat = psmm.tile([1, 3, KC], F32, tag="psmm")
            nc.tensor.matmul(pstat, lhsT=ones_col, rhs=s_t,
                             start=True, stop=True)
            red = stats.tile([1, 3], F32, tag="red")
            nc.vector.reduce_sum(out=red, in_=pstat, axis=AX.X)
            e3 = stats.tile([1, 3], F32, tag="e3")
            nc.vector.tensor_scalar_mul(e3, red, 1.0 / C)
            m2 = stats.tile([1, 1], F32, tag="m2")
            nc.vector.tensor_mul(m2, e3[:, 0:1], e3[:, 0:1])
            var = stats.tile([1, 1], F32, tag="var")
            nc.vector.tensor_add(var, e3[:, 1:2], e3[:, 2:3])
            nc.vector.tensor_sub(var, var, m2)
            std = stats.tile([1, 1], F32, tag="std")
            nc.scalar.activation(out=std, in_=var, func=ACTF.Sqrt,
                                 bias=eps_sb, scale=1.0)
            rstd = stats.tile([1, 1], F32, tag="rstd")
            nc.vector.reciprocal(out=rstd, in_=std)
            sc2 = stats.tile([1, 2], F32, tag="sc2")
            nc.vector.tensor_copy(out=sc2[:, 0:1], in_=e3[:, 0:1])
            nc.vector.tensor_copy(out=sc2[:, 1:2], in_=rstd)
            bc_ps = psmm.tile([P, 2], F32, tag="psmm")
            nc.tensor.matmul(bc_ps, lhsT=ones_row, rhs=sc2,
                             start=True, stop=True)
            # s = gamma * rstd ; t = beta - mean_total * s
            s_ch = stats.tile([P, KC], F32, tag="s_ch")
            nc.vector.tensor_scalar_mul(s_ch, gamma_sb, bc_ps[:, 1:2])
            t_ch = stats.tile([P, KC], F32, tag="t_ch")
            nc.vector.tensor_scalar_mul(t_ch, s_ch, bc_ps[:, 0:1])
            nc.vector.tensor_sub(t_ch, beta_sb, t_ch)
            st_scale[b] = (s_ch, t_ch)

        def emit_tail_apply(b):
            """normalization apply + writeout."""
            y_lo, y_hi = st_y[b]
            s_ch, t_ch = st_scale[b]
            for m in range(KC):
                yt = y_lo if m < 2 else y_hi
                mi = m % 2
                for h in range(2):
                    sl = slice(h * (N // 2), (h + 1) * (N // 2))
                    if m % 2 == 0:
                        nc.vector.tensor_scalar(
                            out=yt[:, mi, sl], in0=yt[:, mi, sl],
                            scalar1=s_ch[:, m:m + 1], scalar2=t_ch[:, m:m + 1],
                            op0=ALU.mult, op1=ALU.add)
                    else:
                        nc.scalar.activation(
                            out=yt[:, mi, sl], in_=yt[:, mi, sl],
                            func=ACTF.Identity,
                            bias=t_ch[:, m:m + 1], scale=s_ch[:, m:m + 1])
                    nc.sync.dma_start(out=out_d[b, m * P:(m + 1) * P, sl],
                                      in_=yt[:, mi, sl])

        emit_A_setup(0)
        emit_A_blocks(0, range(NB))
        emit_softmax(0)
        emit_A_setup(1)
        emit_A_blocks(1, range(2))
        emit_attnv(0)
        emit_By(0)
        emit_A_blocks(1, range(2, NB))
        emit_tail_stats(0)
        emit_softmax(1)
        emit_attnv(1)
        emit_tail_apply(0)
        emit_By(1)
        emit_tail_stats(1)
        emit_tail_apply(1)

    nc.finalize()
    return nc


_NC_CACHE = {}


def _get_nc():
    if "nc" not in _NC_CACHE:
        _NC_CACHE["nc"] = build_nc()
    return _NC_CACHE["nc"]


def _prep_w(w):
    # [C_in, C_out] -> [128, KC, C_out] fp16 with c_in = k*128 + p
    return np.ascontiguousarray(
        w.reshape(KC, P, C).transpose(1, 0, 2).astype(np.float16))


def _prep_vec(v):
    # [C] -> [128, KC] with c = k*128 + p
    return np.ascontiguousarray(v.reshape(KC, P).T)


def _prep_x(x):
    # [B, C, N] -> [B, NB, P, KC*512] fp16: block j, partition p, (k, n)
    nb = np.asarray(x).shape[0]
    xr = np.asarray(x, dtype=np.float32).reshape(nb, KC, P, NB, 512)
    return np.ascontiguousarray(
        xr.transpose(0, 3, 2, 1, 4).astype(np.float16)).reshape(
        nb, NB, P, KC * 512)


def _prep_x_local(x):
    return _prep_x(x)


def _make_in_maps(x, w_qkv, w_out, b_out, gamma, beta):
    xr = _prep_x(x)
    w_qkv = np.asarray(w_qkv, dtype=np.float32)
    wq = _prep_w(np.ascontiguousarray(w_qkv[0:C].T))
    wk = _prep_w(np.ascontiguousarray(w_qkv[C:2 * C].T))
    wv = _prep_w(np.ascontiguousarray(w_qkv[2 * C:3 * C].T))
    wo = _prep_w(np.ascontiguousarray(np.asarray(w_out, dtype=np.float32).T))
    bvec = _prep_vec(np.asarray(b_out, dtype=np.float32))
    gam = _prep_vec(np.asarray(gamma, dtype=np.float32))
    bet = _prep_vec(np.asarray(beta, dtype=np.float32))
    return [
        dict(x=np.ascontiguousarray(xr[c * PB:(c + 1) * PB]),
             wq=wq, wk=wk, wv=wv, wo=wo,
             bvec=bvec, gamma=gam, beta=bet)
        for c in range(NCORES)
    ]


def _run(inputs, trace=False, trace_kwargs=None):
    nc = _get_nc()
    in_maps = _make_in_maps(**inputs)
    res = run_bass_kernel_spmd(nc, in_maps, core_ids=list(range(NCORES)),
                               trace=trace, **(trace_kwargs or {}))
    out = np.concatenate([res.results[c]["out"] for c in range(NCORES)], axis=0)
    return out.reshape(B, C, HW_SIDE, HW_SIDE), res


def kernel(x, w_qkv, w_out, b_out, gamma, beta):
    out, _ = _run(dict(x=x, w_qkv=w_qkv, w_out=w_out, b_out=b_out,
                       gamma=gamma, beta=beta))
    return out


# revision 18
# speedup vs baseline: 1.1687x; 1.0360x over previous
"""Trainium2 Bass kernel for nn_MultiHeadAttention_63814624084186.

Reference computation (per batch sample b, fully independent across b):
  x: [512, 4096]  (C channels x N=64*64 pixels)
  qkv = w_qkv @ x            -> q,k,v each [512, 4096] (8 heads x 64 dims)
  scores = (q_h @ k_h^T)/8   -> [64, 64] per head   (channel-attention)
  attn = softmax(scores, -1)
  out_h = attn_h @ v_h       -> [64, 4096]
  y = w_out @ out + b_out    -> [512, 4096]
  y = groupnorm(y over all C,N) * gamma + beta

Sharding: pure data-parallel over batch: 16 samples / 8 cores = 2 per core.

Design notes:
  - q/k/v GEMMs run in float32r (tf32-class precision at bf16-like speed
    for N=512).  x and w_q/w_k/w_v are DMA'd straight into float32r tiles
    (PE rounds on read; verified on HW).
  - phase 1+2 are n-blocked (8 blocks of 512 pixels): per block we DMA an
    x block (one DMA per channel chunk), compute qT/kT blocks ([N,512]
    layout via GEMM "transpose": lhsT = x block), accumulate scores into
    4 persistent PSUM banks, and compute v for the block.
  - scores/attn@v/out-proj run in bf16 (error contribution ~3e-3).
  - GroupNorm: bn_stats per PSUM tile (bias folded into the cross-
    partition combine), cross-partition reduce via ones-matmul.
  - The two batches are emitted interleaved:
    A(0) B(0) A(1) tail(0) B(1) tail(1), where A = blocked qkv+scores,
    B = softmax/attn@v/out-proj/bn_stats, tail = stat combine+apply+store.
    This hides batch 0's epilogue fully under batch 1's compute and keeps
    the PE queue free of stat matmuls between batches.
  - Weights arrive host-prearranged as [128, KC, C] so weight DMAs are
    contiguous per partition (few descriptors, fast issue).
"""

import numpy as np
from contextlib import ExitStack

import concourse.bass as bass
import concourse.tile as tile
from concourse import bacc, mybir
from concourse.bass_utils import run_bass_kernel_spmd
from concourse.masks import make_identity

F32 = mybir.dt.float32
F32R = mybir.dt.float32r
F16 = mybir.dt.float16
BF16 = mybir.dt.bfloat16
AX = mybir.AxisListType
ALU = mybir.AluOpType
ACTF = mybir.ActivationFunctionType

B = 16          # global batch
C = 512         # channels
N = 4096        # pixels (64*64)
HW_SIDE = 64
NCORES = 8
PB = B // NCORES  # batches per core
P = 128
KC = C // P     # 4 channel chunks
NB = 8          # n blocks of 512
NBI = 4         # 128-chunks per n block
NS = N // 512   # 8 pixel chunks of 512
NHP = 4         # head pairs
EPS = 1e-5


def build_nc():
    nc = bacc.Bacc("TRN2", target_bir_lowering=False, debug=False,
                   num_devices=NCORES)

    x_d = nc.declare_dram_parameter("x", [PB, NB, P, KC * 512], F16, isOutput=False)
    wq_d = nc.declare_dram_parameter("wq", [P, KC, C], F16, isOutput=False)
    wk_d = nc.declare_dram_parameter("wk", [P, KC, C], F16, isOutput=False)
    wv_d = nc.declare_dram_parameter("wv", [P, KC, C], F16, isOutput=False)
    wo_d = nc.declare_dram_parameter("wo", [P, KC, C], F16, isOutput=False)
    bias_d = nc.declare_dram_parameter("bvec", [P, KC], F32, isOutput=False)
    gamma_d = nc.declare_dram_parameter("gamma", [P, KC], F32, isOutput=False)
    beta_d = nc.declare_dram_parameter("beta", [P, KC], F32, isOutput=False)
    out_d = nc.declare_dram_parameter("out", [PB, C, N], F16, isOutput=True)

    with tile.TileContext(nc) as tc, ExitStack() as ctx:
        consts = ctx.enter_context(tc.tile_pool(name="consts", bufs=1))
        xpool = ctx.enter_context(tc.tile_pool(name="xpool", bufs=3))
        qkpool = ctx.enter_context(tc.tile_pool(name="qkpool", bufs=4))
        vpool = ctx.enter_context(tc.tile_pool(name="vpool", bufs=2))
        aopool = ctx.enter_context(tc.tile_pool(name="aopool", bufs=1))
        ypool = ctx.enter_context(tc.tile_pool(name="ypool", bufs=2))
        attn = ctx.enter_context(tc.tile_pool(name="attn", bufs=8))
        attnt = ctx.enter_context(tc.tile_pool(name="attnt", bufs=4))
        stats = ctx.enter_context(tc.tile_pool(name="stats", bufs=2))
        psmm = ctx.enter_context(tc.tile_pool(name="psmm", bufs=4, space="PSUM"))
        pssc = ctx.enter_context(tc.tile_pool(name="pssc", bufs=4, space="PSUM"))

        # ---- prefetch first x blocks before weights (lead-in) ----
        # ---- interleave weight and x-block loads for minimal lead-in ----
        def load_w(dram):
            t = consts.tile([P, KC, C], F16, tag=f"w_{dram.name}")
            nc.sync.dma_start(out=t, in_=dram[:, :, :])
            return t

        prefetched_x = {}

        def prefetch_x(j):
            xt = xpool.tile([P, KC, 512], F16, tag="xblk", name=f"x_0_{j}")
            nc.sync.dma_start(
                out=xt, in_=x_d[0, j].rearrange("p (k n) -> p k n", k=KC))
            prefetched_x[j] = xt

        wq_sb = load_w(wq_d)
        prefetch_x(0)
        wk_sb = load_w(wk_d)
        prefetch_x(1)
        wv_sb = load_w(wv_d)
        prefetch_x(2)
        wo_sb = load_w(wo_d)

        bias_sb = consts.tile([P, KC], F32, tag="bias")
        nc.gpsimd.dma_start(out=bias_sb, in_=bias_d[:, :])
        gamma_sb = consts.tile([P, KC], F32, tag="gamma")
        nc.gpsimd.dma_start(out=gamma_sb, in_=gamma_d[:, :])
        beta_sb = consts.tile([P, KC], F32, tag="beta")
        nc.gpsimd.dma_start(out=beta_sb, in_=beta_d[:, :])

        ident = consts.tile([P, P], F16, tag="ident")
        make_identity(nc, ident)

        eps_sb = consts.tile([1, 1], F32, tag="eps")
        nc.vector.memset(eps_sb, EPS)
        ones_col = consts.tile([P, 1], F32, tag="ones_col")
        nc.vector.memset(ones_col, 1.0)
        ones_row = consts.tile([1, P], F32, tag="ones_row")
        nc.vector.memset(ones_row, 1.0)

        # per-batch state carried between emission stages
        st_v = {}
        st_sc = {}
        st_y = {}
        st_stats = {}

        def emit_A_setup(b):
            v_sb = vpool.tile([P, NHP, N], F16, tag="v", name=f"v_{b}")
            sc_ps = [pssc.tile([P, P], F32, tag="pssc", name=f"sc_{b}_{hp}")
                     for hp in range(NHP)]
            st_v[b] = v_sb
            st_sc[b] = sc_ps

        def emit_A_blocks(b, blocks):
            """n-blocked qT/kT GEMMs, score accumulation, v GEMM."""
            v_sb = st_v[b]
            sc_ps = st_sc[b]
            for j in blocks:
                if b == 0 and j in prefetched_x:
                    x_blk = prefetched_x[j]
                else:
                    x_blk = xpool.tile([P, KC, 512], F16, tag="xblk",
                                       name=f"x_{b}_{j}")
                    nc.sync.dma_start(
                        out=x_blk,
                        in_=x_d[b, j].rearrange("p (k n) -> p k n", k=KC))

                qT_blk = qkpool.tile([P, NBI, C], F16, tag="qk",
                                     name=f"qT_{b}_{j}")
                kT_blk = qkpool.tile([P, NBI, C], F16, tag="qk",
                                     name=f"kT_{b}_{j}")
                for dst, w in ((qT_blk, wq_sb), (kT_blk, wk_sb)):
                    for i in range(NBI):
                        ps = psmm.tile([P, C], F32, tag="psmm")
                        for k in range(KC):
                            nc.tensor.matmul(
                                ps,
                                lhsT=x_blk[:, k, i * P:(i + 1) * P],
                                rhs=w[:, k, :],
                                start=(k == 0), stop=(k == KC - 1))
                        nc.scalar.copy(out=dst[:, i, :], in_=ps)

                for hp in range(NHP):
                    cl = slice(hp * P, (hp + 1) * P)
                    for i in range(NBI):
                        nc.tensor.matmul(
                            sc_ps[hp],
                            lhsT=qT_blk[:, i, cl],
                            rhs=kT_blk[:, i, cl],
                            start=(j == 0 and i == 0),
                            stop=(j == NB - 1 and i == NBI - 1),
                            skip_group_check=True)

                for hp in range(NHP):
                    cl = slice(hp * P, (hp + 1) * P)
                    ps = psmm.tile([P, 512], F32, tag="psmm")
                    for k in range(KC):
                        nc.tensor.matmul(
                            ps,
                            lhsT=wv_sb[:, k, cl],
                            rhs=x_blk[:, k, :],
                            start=(k == 0), stop=(k == KC - 1))
                    nc.vector.tensor_copy(
                        out=v_sb[:, hp, j * 512:(j + 1) * 512], in_=ps)

        st_ao = {}
        st_at = {}

        def emit_softmax(b):
            """softmax on the accumulated score blocks."""
            sc_ps = st_sc[b]
            attnT_tiles = []
            for hp in range(NHP):
                a_sc = attn.tile([P, 64], F32, tag="a_sc")
                nc.vector.tensor_copy(out=a_sc[0:64, :], in_=sc_ps[hp][0:64, 0:64])
                nc.vector.tensor_copy(out=a_sc[64:P, :], in_=sc_ps[hp][64:P, 64:P])
                mx = attn.tile([P, 1], F32, tag="mx")
                nc.vector.reduce_max(out=mx, in_=a_sc, axis=AX.X)
                nmx = attn.tile([P, 1], F32, tag="nmx")
                nc.vector.tensor_scalar_mul(nmx, mx, -0.125)
                a_e = attn.tile([P, 64], F32, tag="a_e")
                nc.scalar.activation(out=a_e, in_=a_sc, func=ACTF.Exp,
                                     bias=nmx, scale=0.125)
                sm = attn.tile([P, 1], F32, tag="sm")
                nc.vector.reduce_sum(out=sm, in_=a_e, axis=AX.X)
                rs = attn.tile([P, 1], F32, tag="rs")
                nc.vector.reciprocal(out=rs, in_=sm)
                a_mm = attn.tile([P, 64], F16, tag="a_mm")
                nc.vector.tensor_scalar_mul(a_mm, a_e, rs)
                at = attnt.tile([P, P], F16, tag="attnT", name=f"at_{b}_{hp}")
                nc.gpsimd.memset(at, 0.0)
                attnT_tiles.append((at, a_mm))
            st_at[b] = attnT_tiles

        def emit_attnv(b):
            """attn transposes + attn @ v."""
            v_sb = st_v[b]
            attnT_tiles = st_at[b]
            ao = aopool.tile([P, KC, N], F16, tag="ao", name=f"ao_{b}")
            for hp in range(NHP):
                at, a_mm = attnT_tiles[hp]
                pt = psmm.tile([P, 64], F16, tag="psmm")
                nc.tensor.transpose(pt[0:64, :], a_mm[0:64, :], ident[0:64, 0:64])
                nc.tensor.transpose(pt[64:P, :], a_mm[64:P, :], ident[64:P, 64:P])
                nc.vector.tensor_copy(out=at[0:64, 0:64], in_=pt[0:64, :])
                nc.vector.tensor_copy(out=at[64:P, 64:P], in_=pt[64:P, :])
                for ns in range(NS):
                    ps = psmm.tile([P, 512], F32, tag="psmm")
                    nc.tensor.matmul(ps, lhsT=at,
                                     rhs=v_sb[:, hp, ns * 512:(ns + 1) * 512],
                                     start=True, stop=True)
                    nc.vector.tensor_copy(out=ao[:, hp, ns * 512:(ns + 1) * 512],
                                          in_=ps)
            st_ao[b] = ao

        def emit_By(b):
            """out projection + bn_stats."""
            ao = st_ao[b]
            y_lo = ypool.tile([P, 2, N], F16, tag="y", name=f"ylo_{b}")
            y_hi = ypool.tile([P, 2, N], F16, tag="y", name=f"yhi_{b}")
            st = stats.tile([P, KC, NS, 6], F32, tag="bnstats")
            st_y[b] = (y_lo, y_hi)
            st_stats[b] = st
            for m in range(KC):
                yt = y_lo if m < 2 else y_hi
                mi = m % 2
                for ns in range(NS):
                    ps = psmm.tile([P, 512], F32, tag="psmm")
                    for k in range(KC):
                        nc.tensor.matmul(
                            ps,
                            lhsT=wo_sb[:, k, m * P:(m + 1) * P],
                            rhs=ao[:, k, ns * 512:(ns + 1) * 512],
                            start=(k == 0), stop=(k == KC - 1))
                    # stats on pre-bias values (bias folded in below)
                    nc.vector.bn_stats(out=st[:, m, ns, :], in_=ps)
                    nc.scalar.add(out=yt[:, mi, ns * 512:(ns + 1) * 512],
                                  in_=ps, add=bias_sb[:, m:m + 1])

        st_scale = {}

        def emit_tail_stats(b):
            """global mean/var combine."""
            st = st_stats[b]
            mv = stats.tile([P, KC, 2], F32, tag="mv")
            for m in range(KC):
                nc.vector.bn_aggr(out=mv[:, m, :], in_=st[:, m])
            # S[p, stat, m]: 0 = mean+bias, 1 = var, 2 = (mean+bias)^2
            s_t = stats.tile([P, 3, KC], F32, tag="s_t")
            nc.vector.tensor_add(s_t[:, 0, :], mv[:, :, 0], bias_sb)
            nc.vector.tensor_copy(out=s_t[:, 1, :], in_=mv[:, :, 1])
            nc.vector.tensor_mul(s_t[:, 2, :], s_t[:, 0, :], s_t[:, 0, :])
            pstat = psmm.tile([1, 3, KC], F32, tag="psmm")
            nc.tensor.matmul(pstat, lhsT=ones_col, rhs=s_t,
                             start=True, stop=True)
            red = stats.tile([1, 3], F32, tag="red")
            nc.vector.reduce_sum(out=red, in_=pstat, axis=AX.X)
            e3 = stats.tile([1, 3], F32, tag="e3")
            nc.vector.tensor_scalar_mul(e3, red, 1.0 / C)
            m2 = stats.tile([1, 1], F32, tag="m2")
            nc.vector.tensor_mul(m2, e3[:, 0:1], e3[:, 0:1])
            var = stats.tile([1, 1], F32, tag="var")
            nc.vector.tensor_add(var, e3[:, 1:2], e3[:, 2:3])
            nc.vector.tensor_sub(var, var, m2)
            std = stats.tile([1, 1], F32, tag="std")
            nc.scalar.activation(out=std, in_=var, func=ACTF.Sqrt,
                                 bias=eps_sb, scale=1.0)
            rstd = stats.tile([1, 1], F32, tag="rstd")
            nc.vector.reciprocal(out=rstd, in_=std)
            sc2 = stats.tile([1, 2], F32, tag="sc2")
            nc.vector.tensor_copy(out=sc2[:, 0:1], in_=e3[:, 0:1])
            nc.vector.tensor_copy(out=sc2[:, 1:2], in_=rstd)
            bc_ps = psmm.tile([P, 2], F32, tag="psmm")
            nc.tensor.matmul(bc_ps, lhsT=ones_row, rhs=sc2,
                             start=True, stop=True)
            # s = gamma * rstd ; t = beta - mean_total * s
            s_ch = stats.tile([P, KC], F32, tag="s_ch")
            nc.vector.tensor_scalar_mul(s_ch, gamma_sb, bc_ps[:, 1:2])
            t_ch = stats.tile([P, KC], F32, tag="t_ch")
            nc.vector.tensor_scalar_mul(t_ch, s_ch, bc_ps[:, 0:1])
            nc.vector.tensor_sub(t_ch, beta_sb, t_ch)
            st_scale[b] = (s_ch, t_ch)

        def emit_tail_apply(b):
            """normalization apply + writeout."""
            y_lo, y_hi = st_y[b]
            s_ch, t_ch = st_scale[b]
            for m in range(KC):
                yt = y_lo if m < 2 else y_hi
                mi = m % 2
                for h in range(2):
                    sl = slice(h * (N // 2), (h + 1) * (N // 2))
                    if m % 2 == 0:
                        nc.vector.tensor_scalar(
                            out=yt[:, mi, sl], in0=yt[:, mi, sl],
                            scalar1=s_ch[:, m:m + 1], scalar2=t_ch[:, m:m + 1],
                            op0=ALU.mult, op1=ALU.add)
                    else:
                        nc.scalar.activation(
                            out=yt[:, mi, sl], in_=yt[:, mi, sl],
                            func=ACTF.Identity,
                            bias=t_ch[:, m:m + 1], scale=s_ch[:, m:m + 1])
                    nc.sync.dma_start(out=out_d[b, m * P:(m + 1) * P, sl],
                                      in_=yt[:, mi, sl])

        emit_A_setup(0)
        emit_A_blocks(0, range(NB))
        emit_softmax(0)
        emit_A_setup(1)
        emit_A_blocks(1, range(2))
        emit_attnv(0)
        emit_By(0)
        emit_A_blocks(1, range(2, NB))
        emit_softmax(1)
        emit_tail_stats(0)
        emit_attnv(1)
        emit_tail_apply(0)
        emit_By(1)
        emit_tail_stats(1)
        emit_tail_apply(1)

    nc.finalize()
    return nc


_NC_CACHE = {}


def _get_nc():
    if "nc" not in _NC_CACHE:
        _NC_CACHE["nc"] = build_nc()
    return _NC_CACHE["nc"]


def _prep_w(w):
    # [C_in, C_out] -> [128, KC, C_out] fp16 with c_in = k*128 + p
    return np.ascontiguousarray(
        w.reshape(KC, P, C).transpose(1, 0, 2).astype(np.float16))


def _prep_vec(v):
    # [C] -> [128, KC] with c = k*128 + p
    return np.ascontiguousarray(v.reshape(KC, P).T)


def _prep_x(x):
    # [B, C, N] -> [B, NB, P, KC*512] fp16: block j, partition p, (k, n)
    nb = np.asarray(x).shape[0]
    xr = np.asarray(x, dtype=np.float32).reshape(nb, KC, P, NB, 512)
    return np.ascontiguousarray(
        xr.transpose(0, 3, 2, 1, 4).astype(np.float16)).reshape(
        nb, NB, P, KC * 512)


def _prep_x_local(x):
    return _prep_x(x)


def _make_in_maps(x, w_qkv, w_out, b_out, gamma, beta):
    xr = _prep_x(x)
    w_qkv = np.asarray(w_qkv, dtype=np.float32)
    wq = _prep_w(np.ascontiguousarray(w_qkv[0:C].T))
    wk = _prep_w(np.ascontiguousarray(w_qkv[C:2 * C].T))
    wv = _prep_w(np.ascontiguousarray(w_qkv[2 * C:3 * C].T))
    wo = _prep_w(np.ascontiguousarray(np.asarray(w_out, dtype=np.float32).T))
    bvec = _prep_vec(np.asarray(b_out, dtype=np.float32))
    gam = _prep_vec(np.asarray(gamma, dtype=np.float32))
    bet = _prep_vec(np.asarray(beta, dtype=np.float32))
    return [
        dict(x=np.ascontiguousarray(xr[c * PB:(c + 1) * PB]),
             wq=wq, wk=wk, wv=wv, wo=wo,
             bvec=bvec, gamma=gam, beta=bet)
        for c in range(NCORES)
    ]


def _run(inputs, trace=False, trace_kwargs=None):
    nc = _get_nc()
    in_maps = _make_in_maps(**inputs)
    res = run_bass_kernel_spmd(nc, in_maps, core_ids=list(range(NCORES)),
                               trace=trace, **(trace_kwargs or {}))
    out = np.concatenate([res.results[c]["out"].astype(np.float32)
                          for c in range(NCORES)], axis=0)
    return out.reshape(B, C, HW_SIDE, HW_SIDE), res


def kernel(x, w_qkv, w_out, b_out, gamma, beta):
    out, _ = _run(dict(x=x, w_qkv=w_qkv, w_out=w_out, b_out=b_out,
                       gamma=gamma, beta=beta))
    return out


# revision 19
# speedup vs baseline: 1.3177x; 1.1276x over previous
"""Trainium2 Bass kernel for nn_MultiHeadAttention_63814624084186.

Reference computation (per batch sample b, fully independent across b):
  x: [512, 4096]  (C channels x N=64*64 pixels)
  qkv = w_qkv @ x            -> q,k,v each [512, 4096] (8 heads x 64 dims)
  scores = (q_h @ k_h^T)/8   -> [64, 64] per head   (channel-attention)
  attn = softmax(scores, -1)
  out_h = attn_h @ v_h       -> [64, 4096]
  y = w_out @ out + b_out    -> [512, 4096]
  y = groupnorm(y over all C,N) * gamma + beta

Sharding: pure data-parallel over batch: 16 samples / 8 cores = 2 per core.

Design notes:
  - q/k/v GEMMs run in float32r (tf32-class precision at bf16-like speed
    for N=512).  x and w_q/w_k/w_v are DMA'd straight into float32r tiles
    (PE rounds on read; verified on HW).
  - phase 1+2 are n-blocked (8 blocks of 512 pixels): per block we DMA an
    x block (one DMA per channel chunk), compute qT/kT blocks ([N,512]
    layout via GEMM "transpose": lhsT = x block), accumulate scores into
    4 persistent PSUM banks, and compute v for the block.
  - scores/attn@v/out-proj run in bf16 (error contribution ~3e-3).
  - GroupNorm: bn_stats per PSUM tile (bias folded into the cross-
    partition combine), cross-partition reduce via ones-matmul.
  - The two batches are emitted interleaved:
    A(0) B(0) A(1) tail(0) B(1) tail(1), where A = blocked qkv+scores,
    B = softmax/attn@v/out-proj/bn_stats, tail = stat combine+apply+store.
    This hides batch 0's epilogue fully under batch 1's compute and keeps
    the PE queue free of stat matmuls between batches.
  - Weights arrive host-prearranged as [128, KC, C] so weight DMAs are
    contiguous per partition (few descriptors, fast issue).
"""

import numpy as np
from contextlib import ExitStack

import concourse.bass as bass
import concourse.tile as tile
from concourse import bacc, mybir
from concourse.bass_utils import run_bass_kernel_spmd

F32 = mybir.dt.float32
F32R = mybir.dt.float32r
F16 = mybir.dt.float16
BF16 = mybir.dt.bfloat16
AX = mybir.AxisListType
ALU = mybir.AluOpType
ACTF = mybir.ActivationFunctionType

B = 16          # global batch
C = 512         # channels
N = 4096        # pixels (64*64)
HW_SIDE = 64
NCORES = 8
PB = B // NCORES  # batches per core
P = 128
KC = C // P     # 4 channel chunks
NB = 8          # n blocks of 512
NBI = 4         # 128-chunks per n block
NS = N // 512   # 8 pixel chunks of 512
NHP = 4         # head pairs
EPS = 1e-5


def build_nc():
    nc = bacc.Bacc("TRN2", target_bir_lowering=False, debug=False,
                   num_devices=NCORES)

    x_d = nc.declare_dram_parameter("x", [PB, NB, P, KC * 512], F16, isOutput=False)
    wq_d = nc.declare_dram_parameter("wq", [P, KC, C], F16, isOutput=False)
    wk_d = nc.declare_dram_parameter("wk", [P, KC, C], F16, isOutput=False)
    wv_d = nc.declare_dram_parameter("wv", [P, KC, C], F16, isOutput=False)
    wo_d = nc.declare_dram_parameter("wo", [P, KC, C], F16, isOutput=False)
    bias_d = nc.declare_dram_parameter("bvec", [P, KC], F32, isOutput=False)
    gamma_d = nc.declare_dram_parameter("gamma", [P, KC], F32, isOutput=False)
    beta_d = nc.declare_dram_parameter("beta", [P, KC], F32, isOutput=False)
    out_d = nc.declare_dram_parameter("out", [PB, C, N], F16, isOutput=True)

    with tile.TileContext(nc) as tc, ExitStack() as ctx:
        consts = ctx.enter_context(tc.tile_pool(name="consts", bufs=1))
        xpool = ctx.enter_context(tc.tile_pool(name="xpool", bufs=3))
        qkpool = ctx.enter_context(tc.tile_pool(name="qkpool", bufs=4))
        vpool = ctx.enter_context(tc.tile_pool(name="vpool", bufs=2))
        w2pool = ctx.enter_context(tc.tile_pool(name="w2pool", bufs=2))
        ypool = ctx.enter_context(tc.tile_pool(name="ypool", bufs=2))
        attn = ctx.enter_context(tc.tile_pool(name="attn", bufs=8))
        attnt = ctx.enter_context(tc.tile_pool(name="attnt", bufs=4))
        stats = ctx.enter_context(tc.tile_pool(name="stats", bufs=2))
        psmm = ctx.enter_context(tc.tile_pool(name="psmm", bufs=4, space="PSUM"))
        pssc = ctx.enter_context(tc.tile_pool(name="pssc", bufs=4, space="PSUM"))

        # ---- prefetch first x blocks before weights (lead-in) ----
        # ---- interleave weight and x-block loads for minimal lead-in ----
        def load_w(dram):
            t = consts.tile([P, KC, C], F16, tag=f"w_{dram.name}")
            nc.sync.dma_start(out=t, in_=dram[:, :, :])
            return t

        prefetched_x = {}

        def prefetch_x(j):
            xt = xpool.tile([P, KC, 512], F16, tag="xblk", name=f"x_0_{j}")
            nc.sync.dma_start(
                out=xt, in_=x_d[0, j].rearrange("p (k n) -> p k n", k=KC))
            prefetched_x[j] = xt

        wq_sb = load_w(wq_d)
        prefetch_x(0)
        wk_sb = load_w(wk_d)
        prefetch_x(1)
        wv_sb = load_w(wv_d)
        prefetch_x(2)
        wo_sb = load_w(wo_d)

        bias_sb = consts.tile([P, KC], F32, tag="bias")
        nc.gpsimd.dma_start(out=bias_sb, in_=bias_d[:, :])
        gamma_sb = consts.tile([P, KC], F32, tag="gamma")
        nc.gpsimd.dma_start(out=gamma_sb, in_=gamma_d[:, :])
        beta_sb = consts.tile([P, KC], F32, tag="beta")
        nc.gpsimd.dma_start(out=beta_sb, in_=beta_d[:, :])

        eps_sb = consts.tile([1, 1], F32, tag="eps")
        nc.vector.memset(eps_sb, EPS)
        ones_col = consts.tile([P, 1], F32, tag="ones_col")
        nc.vector.memset(ones_col, 1.0)
        ones_row = consts.tile([1, P], F32, tag="ones_row")
        nc.vector.memset(ones_row, 1.0)

        # per-batch state carried between emission stages
        st_v = {}
        st_sc = {}
        st_y = {}
        st_stats = {}

        def emit_A_setup(b):
            v_sb = vpool.tile([P, NHP, N], F16, tag="v", name=f"v_{b}")
            sc_ps = [pssc.tile([P, P], F32, tag="pssc", name=f"sc_{b}_{hp}")
                     for hp in range(NHP)]
            st_v[b] = v_sb
            st_sc[b] = sc_ps

        def emit_A_blocks(b, blocks):
            """n-blocked qT/kT GEMMs, score accumulation, v GEMM."""
            v_sb = st_v[b]
            sc_ps = st_sc[b]
            for j in blocks:
                if b == 0 and j in prefetched_x:
                    x_blk = prefetched_x[j]
                else:
                    x_blk = xpool.tile([P, KC, 512], F16, tag="xblk",
                                       name=f"x_{b}_{j}")
                    nc.sync.dma_start(
                        out=x_blk,
                        in_=x_d[b, j].rearrange("p (k n) -> p k n", k=KC))

                qT_blk = qkpool.tile([P, NBI, C], F16, tag="qk",
                                     name=f"qT_{b}_{j}")
                kT_blk = qkpool.tile([P, NBI, C], F16, tag="qk",
                                     name=f"kT_{b}_{j}")
                for dst, w in ((qT_blk, wq_sb), (kT_blk, wk_sb)):
                    for i in range(NBI):
                        ps = psmm.tile([P, C], F32, tag="psmm")
                        for k in range(KC):
                            nc.tensor.matmul(
                                ps,
                                lhsT=x_blk[:, k, i * P:(i + 1) * P],
                                rhs=w[:, k, :],
                                start=(k == 0), stop=(k == KC - 1))
                        nc.scalar.copy(out=dst[:, i, :], in_=ps)

                for hp in range(NHP):
                    cl = slice(hp * P, (hp + 1) * P)
                    for i in range(NBI):
                        nc.tensor.matmul(
                            sc_ps[hp],
                            lhsT=qT_blk[:, i, cl],
                            rhs=kT_blk[:, i, cl],
                            start=(j == 0 and i == 0),
                            stop=(j == NB - 1 and i == NBI - 1),
                            skip_group_check=True)

                for hp in range(NHP):
                    cl = slice(hp * P, (hp + 1) * P)
                    ps = psmm.tile([P, 512], F32, tag="psmm")
                    for k in range(KC):
                        nc.tensor.matmul(
                            ps,
                            lhsT=wv_sb[:, k, cl],
                            rhs=x_blk[:, k, :],
                            start=(k == 0), stop=(k == KC - 1))
                    nc.vector.tensor_copy(
                        out=v_sb[:, hp, j * 512:(j + 1) * 512], in_=ps)

        st_ao = {}
        st_at = {}

        def emit_softmax(b):
            """softmax on the accumulated score blocks."""
            sc_ps = st_sc[b]
            attnT_tiles = []
            for hp in range(NHP):
                a_sc = attn.tile([P, 64], F32, tag="a_sc")
                nc.vector.tensor_copy(out=a_sc[0:64, :], in_=sc_ps[hp][0:64, 0:64])
                nc.vector.tensor_copy(out=a_sc[64:P, :], in_=sc_ps[hp][64:P, 64:P])
                mx = attn.tile([P, 1], F32, tag="mx")
                nc.vector.reduce_max(out=mx, in_=a_sc, axis=AX.X)
                nmx = attn.tile([P, 1], F32, tag="nmx")
                nc.vector.tensor_scalar_mul(nmx, mx, -0.125)
                a_e = attn.tile([P, 64], F32, tag="a_e")
                nc.scalar.activation(out=a_e, in_=a_sc, func=ACTF.Exp,
                                     bias=nmx, scale=0.125)
                sm = attn.tile([P, 1], F32, tag="sm")
                nc.vector.reduce_sum(out=sm, in_=a_e, axis=AX.X)
                rs = attn.tile([P, 1], F32, tag="rs")
                nc.vector.reciprocal(out=rs, in_=sm)
                a_mm = attn.tile([P, 64], F16, tag="a_mm")
                nc.vector.tensor_scalar_mul(a_mm, a_e, rs)
                at = attnt.tile([P, P], F16, tag="attnT", name=f"at_{b}_{hp}")
                nc.gpsimd.memset(at, 0.0)
                attnT_tiles.append((at, a_mm))
            st_at[b] = attnT_tiles

        def emit_W2(b):
            """fold attn into the out-projection: W2 = blockdiag(A)^T @ woT."""
            attnT_tiles = st_at[b]
            w2 = w2pool.tile([P, KC, C], F16, tag="w2", name=f"w2_{b}")
            for hp in range(NHP):
                at, a_mm = attnT_tiles[hp]
                # block-diagonal attn (untransposed): out = A^T @ woT rows
                nc.vector.tensor_copy(out=at[0:64, 0:64], in_=a_mm[0:64, :])
                nc.vector.tensor_copy(out=at[64:P, 64:P], in_=a_mm[64:P, :])
                ps = psmm.tile([P, C], F32, tag="psmm")
                nc.tensor.matmul(ps, lhsT=at, rhs=wo_sb[:, hp, :],
                                 start=True, stop=True)
                nc.vector.tensor_copy(out=w2[:, hp, :], in_=ps)
            st_ao[b] = w2

        def emit_By(b):
            """out projection (fused weights) + bn_stats."""
            w2 = st_ao[b]
            v_sb = st_v[b]
            y_lo = ypool.tile([P, 2, N], F16, tag="y", name=f"ylo_{b}")
            y_hi = ypool.tile([P, 2, N], F16, tag="y", name=f"yhi_{b}")
            st = stats.tile([P, KC, NS, 6], F32, tag="bnstats")
            st_y[b] = (y_lo, y_hi)
            st_stats[b] = st
            for m in range(KC):
                yt = y_lo if m < 2 else y_hi
                mi = m % 2
                for ns in range(NS):
                    ps = psmm.tile([P, 512], F32, tag="psmm")
                    for k in range(KC):
                        nc.tensor.matmul(
                            ps,
                            lhsT=w2[:, k, m * P:(m + 1) * P],
                            rhs=v_sb[:, k, ns * 512:(ns + 1) * 512],
                            start=(k == 0), stop=(k == KC - 1))
                    # stats on pre-bias values (bias folded in below)
                    nc.vector.bn_stats(out=st[:, m, ns, :], in_=ps)
                    nc.scalar.add(out=yt[:, mi, ns * 512:(ns + 1) * 512],
                                  in_=ps, add=bias_sb[:, m:m + 1])

        st_scale = {}

        def emit_tail_stats(b):
            """global mean/var combine."""
            st = st_stats[b]
            mv = stats.tile([P, KC, 2], F32, tag="mv")
            for m in range(KC):
                nc.vector.bn_aggr(out=mv[:, m, :], in_=st[:, m])
            # S[p, stat, m]: 0 = mean+bias, 1 = var, 2 = (mean+bias)^2
            s_t = stats.tile([P, 3, KC], F32, tag="s_t")
            nc.vector.tensor_add(s_t[:, 0, :], mv[:, :, 0], bias_sb)
            nc.vector.tensor_copy(out=s_t[:, 1, :], in_=mv[:, :, 1])
            nc.vector.tensor_mul(s_t[:, 2, :], s_t[:, 0, :], s_t[:, 0, :])
            pstat = psmm.tile([1, 3, KC], F32, tag="psmm")
            nc.tensor.matmul(pstat, lhsT=ones_col, rhs=s_t,
                             start=True, stop=True)
            red = stats.tile([1, 3], F32, tag="red")
            nc.vector.reduce_sum(out=red, in_=pstat, axis=AX.X)
            e3 = stats.tile([1, 3], F32, tag="e3")
            nc.vector.tensor_scalar_mul(e3, red, 1.0 / C)
            m2 = stats.tile([1, 1], F32, tag="m2")
            nc.vector.tensor_mul(m2, e3[:, 0:1], e3[:, 0:1])
            var = stats.tile([1, 1], F32, tag="var")
            nc.vector.tensor_add(var, e3[:, 1:2], e3[:, 2:3])
            nc.vector.tensor_sub(var, var, m2)
            std = stats.tile([1, 1], F32, tag="std")
            nc.scalar.activation(out=std, in_=var, func=ACTF.Sqrt,
                                 bias=eps_sb, scale=1.0)
            rstd = stats.tile([1, 1], F32, tag="rstd")
            nc.vector.reciprocal(out=rstd, in_=std)
            sc2 = stats.tile([1, 2], F32, tag="sc2")
            nc.vector.tensor_copy(out=sc2[:, 0:1], in_=e3[:, 0:1])
            nc.vector.tensor_copy(out=sc2[:, 1:2], in_=rstd)
            bc_ps = psmm.tile([P, 2], F32, tag="psmm")
            nc.tensor.matmul(bc_ps, lhsT=ones_row, rhs=sc2,
                             start=True, stop=True)
            # s = gamma * rstd ; t = beta - mean_total * s
            s_ch = stats.tile([P, KC], F32, tag="s_ch")
            nc.vector.tensor_scalar_mul(s_ch, gamma_sb, bc_ps[:, 1:2])
            t_ch = stats.tile([P, KC], F32, tag="t_ch")
            nc.vector.tensor_scalar_mul(t_ch, s_ch, bc_ps[:, 0:1])
            nc.vector.tensor_sub(t_ch, beta_sb, t_ch)
            st_scale[b] = (s_ch, t_ch)

        def emit_tail_apply(b):
            """normalization apply + writeout."""
            y_lo, y_hi = st_y[b]
            s_ch, t_ch = st_scale[b]
            for m in range(KC):
                yt = y_lo if m < 2 else y_hi
                mi = m % 2
                for h in range(2):
                    sl = slice(h * (N // 2), (h + 1) * (N // 2))
                    if m % 2 == 0:
                        nc.vector.tensor_scalar(
                            out=yt[:, mi, sl], in0=yt[:, mi, sl],
                            scalar1=s_ch[:, m:m + 1], scalar2=t_ch[:, m:m + 1],
                            op0=ALU.mult, op1=ALU.add)
                    else:
                        nc.scalar.activation(
                            out=yt[:, mi, sl], in_=yt[:, mi, sl],
                            func=ACTF.Identity,
                            bias=t_ch[:, m:m + 1], scale=s_ch[:, m:m + 1])
                    nc.sync.dma_start(out=out_d[b, m * P:(m + 1) * P, sl],
                                      in_=yt[:, mi, sl])

        emit_A_setup(0)
        emit_A_blocks(0, range(NB))
        emit_softmax(0)
        emit_A_setup(1)
        emit_A_blocks(1, range(2))
        emit_W2(0)
        emit_By(0)
        emit_A_blocks(1, range(2, NB))
        emit_softmax(1)
        emit_tail_stats(0)
        emit_W2(1)
        emit_tail_apply(0)
        emit_By(1)
        emit_tail_stats(1)
        emit_tail_apply(1)

    nc.finalize()
    return nc


_NC_CACHE = {}


def _get_nc():
    if "nc" not in _NC_CACHE:
        _NC_CACHE["nc"] = build_nc()
    return _NC_CACHE["nc"]


def _prep_w(w):
    # [C_in, C_out] -> [128, KC, C_out] fp16 with c_in = k*128 + p
    return np.ascontiguousarray(
        w.reshape(KC, P, C).transpose(1, 0, 2).astype(np.float16))


def _prep_vec(v):
    # [C] -> [128, KC] with c = k*128 + p
    return np.ascontiguousarray(v.reshape(KC, P).T)


def _prep_x(x):
    # [B, C, N] -> [B, NB, P, KC*512] fp16: block j, partition p, (k, n)
    nb = np.asarray(x).shape[0]
    xr = np.asarray(x, dtype=np.float32).reshape(nb, KC, P, NB, 512)
    return np.ascontiguousarray(
        xr.transpose(0, 3, 2, 1, 4).astype(np.float16)).reshape(
        nb, NB, P, KC * 512)


def _prep_x_local(x):
    return _prep_x(x)


def _make_in_maps(x, w_qkv, w_out, b_out, gamma, beta):
    xr = _prep_x(x)
    w_qkv = np.asarray(w_qkv, dtype=np.float32)
    wq = _prep_w(np.ascontiguousarray(w_qkv[0:C].T))
    wk = _prep_w(np.ascontiguousarray(w_qkv[C:2 * C].T))
    wv = _prep_w(np.ascontiguousarray(w_qkv[2 * C:3 * C].T))
    wo = _prep_w(np.ascontiguousarray(np.asarray(w_out, dtype=np.float32).T))
    bvec = _prep_vec(np.asarray(b_out, dtype=np.float32))
    gam = _prep_vec(np.asarray(gamma, dtype=np.float32))
    bet = _prep_vec(np.asarray(beta, dtype=np.float32))
    return [
        dict(x=np.ascontiguousarray(xr[c * PB:(c + 1) * PB]),
             wq=wq, wk=wk, wv=wv, wo=wo,
             bvec=bvec, gamma=gam, beta=bet)
        for c in range(NCORES)
    ]


def _run(inputs, trace=False, trace_kwargs=None):
    nc = _get_nc()
    in_maps = _make_in_maps(**inputs)
    res = run_bass_kernel_spmd(nc, in_maps, core_ids=list(range(NCORES)),
                               trace=trace, **(trace_kwargs or {}))
    out = np.concatenate([res.results[c]["out"].astype(np.float32)
                          for c in range(NCORES)], axis=0)
    return out.reshape(B, C, HW_SIDE, HW_SIDE), res


def kernel(x, w_qkv, w_out, b_out, gamma, beta):
    out, _ = _run(dict(x=x, w_qkv=w_qkv, w_out=w_out, b_out=b_out,
                       gamma=gamma, beta=beta))
    return out


# revision 20
# speedup vs baseline: 1.3239x; 1.0047x over previous
"""Trainium2 Bass kernel for nn_MultiHeadAttention_63814624084186.

Reference computation (per batch sample b, fully independent across b):
  x: [512, 4096]  (C channels x N=64*64 pixels)
  qkv = w_qkv @ x            -> q,k,v each [512, 4096] (8 heads x 64 dims)
  scores = (q_h @ k_h^T)/8   -> [64, 64] per head   (channel-attention)
  attn = softmax(scores, -1)
  out_h = attn_h @ v_h       -> [64, 4096]
  y = w_out @ out + b_out    -> [512, 4096]
  y = groupnorm(y over all C,N) * gamma + beta

Sharding: pure data-parallel over batch: 16 samples / 8 cores = 2 per core.

Design notes:
  - q/k/v GEMMs run in float32r (tf32-class precision at bf16-like speed
    for N=512).  x and w_q/w_k/w_v are DMA'd straight into float32r tiles
    (PE rounds on read; verified on HW).
  - phase 1+2 are n-blocked (8 blocks of 512 pixels): per block we DMA an
    x block (one DMA per channel chunk), compute qT/kT blocks ([N,512]
    layout via GEMM "transpose": lhsT = x block), accumulate scores into
    4 persistent PSUM banks, and compute v for the block.
  - scores/attn@v/out-proj run in bf16 (error contribution ~3e-3).
  - GroupNorm: bn_stats per PSUM tile (bias folded into the cross-
    partition combine), cross-partition reduce via ones-matmul.
  - The two batches are emitted interleaved:
    A(0) B(0) A(1) tail(0) B(1) tail(1), where A = blocked qkv+scores,
    B = softmax/attn@v/out-proj/bn_stats, tail = stat combine+apply+store.
    This hides batch 0's epilogue fully under batch 1's compute and keeps
    the PE queue free of stat matmuls between batches.
  - Weights arrive host-prearranged as [128, KC, C] so weight DMAs are
    contiguous per partition (few descriptors, fast issue).
"""

import numpy as np
from contextlib import ExitStack

import concourse.bass as bass
import concourse.tile as tile
from concourse import bacc, mybir
from concourse.bass_utils import run_bass_kernel_spmd

F32 = mybir.dt.float32
F32R = mybir.dt.float32r
F16 = mybir.dt.float16
BF16 = mybir.dt.bfloat16
AX = mybir.AxisListType
ALU = mybir.AluOpType
ACTF = mybir.ActivationFunctionType

B = 16          # global batch
C = 512         # channels
N = 4096        # pixels (64*64)
HW_SIDE = 64
NCORES = 8
PB = B // NCORES  # batches per core
P = 128
KC = C // P     # 4 channel chunks
NB = 8          # n blocks of 512
NBI = 4         # 128-chunks per n block
NS = N // 512   # 8 pixel chunks of 512
NHP = 4         # head pairs
EPS = 1e-5


def build_nc():
    nc = bacc.Bacc("TRN2", target_bir_lowering=False, debug=False,
                   num_devices=NCORES)

    x_d = nc.declare_dram_parameter("x", [PB, NB, P, KC * 512], F16, isOutput=False)
    wq_d = nc.declare_dram_parameter("wq", [P, KC, C], F16, isOutput=False)
    wk_d = nc.declare_dram_parameter("wk", [P, KC, C], F16, isOutput=False)
    wv_d = nc.declare_dram_parameter("wv", [P, KC, C], F16, isOutput=False)
    wo_d = nc.declare_dram_parameter("wo", [P, KC, C], F16, isOutput=False)
    bias_d = nc.declare_dram_parameter("bvec", [P, KC], F32, isOutput=False)
    gamma_d = nc.declare_dram_parameter("gamma", [P, KC], F32, isOutput=False)
    beta_d = nc.declare_dram_parameter("beta", [P, KC], F32, isOutput=False)
    out_d = nc.declare_dram_parameter("out", [PB, C, N], F16, isOutput=True)

    with tile.TileContext(nc) as tc, ExitStack() as ctx:
        consts = ctx.enter_context(tc.tile_pool(name="consts", bufs=1))
        xpool = ctx.enter_context(tc.tile_pool(name="xpool", bufs=3))
        qkpool = ctx.enter_context(tc.tile_pool(name="qkpool", bufs=4))
        vpool = ctx.enter_context(tc.tile_pool(name="vpool", bufs=2))
        w2pool = ctx.enter_context(tc.tile_pool(name="w2pool", bufs=2))
        ypool = ctx.enter_context(tc.tile_pool(name="ypool", bufs=2))
        attn = ctx.enter_context(tc.tile_pool(name="attn", bufs=8))
        attnt = ctx.enter_context(tc.tile_pool(name="attnt", bufs=4))
        stats = ctx.enter_context(tc.tile_pool(name="stats", bufs=2))
        psmm = ctx.enter_context(tc.tile_pool(name="psmm", bufs=4, space="PSUM"))
        pssc = ctx.enter_context(tc.tile_pool(name="pssc", bufs=4, space="PSUM"))

        # ---- prefetch first x blocks before weights (lead-in) ----
        # ---- interleave weight and x-block loads for minimal lead-in ----
        def load_w(dram):
            t = consts.tile([P, KC, C], F16, tag=f"w_{dram.name}")
            nc.sync.dma_start(out=t, in_=dram[:, :, :])
            return t

        prefetched_x = {}

        def prefetch_x(j):
            xt = xpool.tile([P, KC, 512], F16, tag="xblk", name=f"x_0_{j}")
            nc.sync.dma_start(
                out=xt, in_=x_d[0, j].rearrange("p (k n) -> p k n", k=KC))
            prefetched_x[j] = xt

        wq_sb = load_w(wq_d)
        prefetch_x(0)
        wk_sb = load_w(wk_d)
        prefetch_x(1)
        wv_sb = load_w(wv_d)
        prefetch_x(2)
        wo_sb = load_w(wo_d)

        bias_sb = consts.tile([P, KC], F32, tag="bias")
        nc.gpsimd.dma_start(out=bias_sb, in_=bias_d[:, :])
        gamma_sb = consts.tile([P, KC], F32, tag="gamma")
        nc.gpsimd.dma_start(out=gamma_sb, in_=gamma_d[:, :])
        beta_sb = consts.tile([P, KC], F32, tag="beta")
        nc.gpsimd.dma_start(out=beta_sb, in_=beta_d[:, :])

        eps_sb = consts.tile([1, 1], F32, tag="eps")
        nc.vector.memset(eps_sb, EPS)
        ones_col = consts.tile([P, 1], F32, tag="ones_col")
        nc.vector.memset(ones_col, 1.0)
        ones_row = consts.tile([1, P], F32, tag="ones_row")
        nc.vector.memset(ones_row, 1.0)

        # per-batch state carried between emission stages
        st_v = {}
        st_sc = {}
        st_y = {}
        st_stats = {}

        def emit_A_setup(b):
            v_sb = vpool.tile([P, NHP, N], F16, tag="v", name=f"v_{b}")
            sc_ps = [pssc.tile([P, P], F32, tag="pssc", name=f"sc_{b}_{hp}")
                     for hp in range(NHP)]
            st_v[b] = v_sb
            st_sc[b] = sc_ps

        def emit_A_blocks(b, blocks):
            """n-blocked qT/kT GEMMs, score accumulation, v GEMM."""
            v_sb = st_v[b]
            sc_ps = st_sc[b]
            for j in blocks:
                if b == 0 and j in prefetched_x:
                    x_blk = prefetched_x[j]
                else:
                    x_blk = xpool.tile([P, KC, 512], F16, tag="xblk",
                                       name=f"x_{b}_{j}")
                    nc.sync.dma_start(
                        out=x_blk,
                        in_=x_d[b, j].rearrange("p (k n) -> p k n", k=KC))

                qT_blk = qkpool.tile([P, NBI, C], F16, tag="qk",
                                     name=f"qT_{b}_{j}")
                kT_blk = qkpool.tile([P, NBI, C], F16, tag="qk",
                                     name=f"kT_{b}_{j}")
                for dst, w in ((qT_blk, wq_sb), (kT_blk, wk_sb)):
                    for i in range(NBI):
                        ps = psmm.tile([P, C], F32, tag="psmm")
                        for k in range(KC):
                            nc.tensor.matmul(
                                ps,
                                lhsT=x_blk[:, k, i * P:(i + 1) * P],
                                rhs=w[:, k, :],
                                start=(k == 0), stop=(k == KC - 1))
                        nc.scalar.copy(out=dst[:, i, :], in_=ps)

                for hp in range(NHP):
                    cl = slice(hp * P, (hp + 1) * P)
                    for i in range(NBI):
                        nc.tensor.matmul(
                            sc_ps[hp],
                            lhsT=qT_blk[:, i, cl],
                            rhs=kT_blk[:, i, cl],
                            start=(j == 0 and i == 0),
                            stop=(j == NB - 1 and i == NBI - 1),
                            skip_group_check=True)

                for hp in range(NHP):
                    cl = slice(hp * P, (hp + 1) * P)
                    ps = psmm.tile([P, 512], F32, tag="psmm")
                    for k in range(KC):
                        nc.tensor.matmul(
                            ps,
                            lhsT=wv_sb[:, k, cl],
                            rhs=x_blk[:, k, :],
                            start=(k == 0), stop=(k == KC - 1))
                    nc.vector.tensor_copy(
                        out=v_sb[:, hp, j * 512:(j + 1) * 512], in_=ps)

        st_ao = {}
        st_at = {}

        def emit_softmax(b):
            """softmax on the accumulated score blocks (all pairs batched)."""
            sc_ps = st_sc[b]
            a_all = attn.tile([P, NHP, 64], F32, tag="a_all")
            for hp in range(NHP):
                nc.vector.tensor_copy(out=a_all[0:64, hp, :],
                                      in_=sc_ps[hp][0:64, 0:64])
                nc.vector.tensor_copy(out=a_all[64:P, hp, :],
                                      in_=sc_ps[hp][64:P, 64:P])
            mx = attn.tile([P, NHP, 1], F32, tag="mx4")
            nc.vector.reduce_max(out=mx, in_=a_all, axis=AX.X)
            d_all = attn.tile([P, NHP, 64], F32, tag="d_all")
            nc.vector.tensor_tensor(d_all, a_all,
                                    mx.to_broadcast([P, NHP, 64]), ALU.subtract)
            e_all = attn.tile([P, NHP, 64], F32, tag="e_all")
            nc.scalar.activation(out=e_all, in_=d_all, func=ACTF.Exp,
                                 bias=0.0, scale=0.125)
            sm = attn.tile([P, NHP, 1], F32, tag="sm4")
            nc.vector.reduce_sum(out=sm, in_=e_all, axis=AX.X)
            rs = attn.tile([P, NHP, 1], F32, tag="rs4")
            nc.vector.reciprocal(out=rs, in_=sm)
            a_mm = attn.tile([P, NHP, 64], F16, tag="amm4")
            nc.vector.tensor_tensor(a_mm, e_all,
                                    rs.to_broadcast([P, NHP, 64]), ALU.mult)
            attnT_tiles = []
            for hp in range(NHP):
                at = attnt.tile([P, P], F16, tag="attnT", name=f"at_{b}_{hp}")
                nc.gpsimd.memset(at, 0.0)
                attnT_tiles.append((at, a_mm[:, hp, :]))
            st_at[b] = attnT_tiles

        def emit_W2(b):
            """fold attn into the out-projection: W2 = blockdiag(A)^T @ woT."""
            attnT_tiles = st_at[b]
            w2 = w2pool.tile([P, KC, C], F16, tag="w2", name=f"w2_{b}")
            for hp in range(NHP):
                at, a_mm = attnT_tiles[hp]
                # block-diagonal attn (untransposed): out = A^T @ woT rows
                nc.vector.tensor_copy(out=at[0:64, 0:64], in_=a_mm[0:64, :])
                nc.vector.tensor_copy(out=at[64:P, 64:P], in_=a_mm[64:P, :])
                ps = psmm.tile([P, C], F32, tag="psmm")
                nc.tensor.matmul(ps, lhsT=at, rhs=wo_sb[:, hp, :],
                                 start=True, stop=True)
                nc.vector.tensor_copy(out=w2[:, hp, :], in_=ps)
            st_ao[b] = w2

        def emit_By(b):
            """out projection (fused weights) + bn_stats."""
            w2 = st_ao[b]
            v_sb = st_v[b]
            y_lo = ypool.tile([P, 2, N], F16, tag="y", name=f"ylo_{b}")
            y_hi = ypool.tile([P, 2, N], F16, tag="y", name=f"yhi_{b}")
            st = stats.tile([P, KC, NS, 6], F32, tag="bnstats")
            st_y[b] = (y_lo, y_hi)
            st_stats[b] = st
            for m in range(KC):
                yt = y_lo if m < 2 else y_hi
                mi = m % 2
                for ns in range(NS):
                    ps = psmm.tile([P, 512], F32, tag="psmm")
                    for k in range(KC):
                        nc.tensor.matmul(
                            ps,
                            lhsT=w2[:, k, m * P:(m + 1) * P],
                            rhs=v_sb[:, k, ns * 512:(ns + 1) * 512],
                            start=(k == 0), stop=(k == KC - 1))
                    # stats on pre-bias values (bias folded in below)
                    nc.vector.bn_stats(out=st[:, m, ns, :], in_=ps)
                    nc.scalar.add(out=yt[:, mi, ns * 512:(ns + 1) * 512],
                                  in_=ps, add=bias_sb[:, m:m + 1])

        st_scale = {}

        def emit_tail_stats(b):
            """global mean/var combine."""
            st = st_stats[b]
            mv = stats.tile([P, KC, 2], F32, tag="mv")
            for m in range(KC):
                nc.vector.bn_aggr(out=mv[:, m, :], in_=st[:, m])
            # S[p, stat, m]: 0 = mean+bias, 1 = var, 2 = (mean+bias)^2
            s_t = stats.tile([P, 3, KC], F32, tag="s_t")
            nc.vector.tensor_add(s_t[:, 0, :], mv[:, :, 0], bias_sb)
            nc.vector.tensor_copy(out=s_t[:, 1, :], in_=mv[:, :, 1])
            nc.vector.tensor_mul(s_t[:, 2, :], s_t[:, 0, :], s_t[:, 0, :])
            pstat = psmm.tile([1, 3, KC], F32, tag="psmm")
            nc.tensor.matmul(pstat, lhsT=ones_col, rhs=s_t,
                             start=True, stop=True)
            red = stats.tile([1, 3], F32, tag="red")
            nc.vector.reduce_sum(out=red, in_=pstat, axis=AX.X)
            e3 = stats.tile([1, 3], F32, tag="e3")
            nc.vector.tensor_scalar_mul(e3, red, 1.0 / C)
            m2 = stats.tile([1, 1], F32, tag="m2")
            nc.vector.tensor_mul(m2, e3[:, 0:1], e3[:, 0:1])
            var = stats.tile([1, 1], F32, tag="var")
            nc.vector.tensor_add(var, e3[:, 1:2], e3[:, 2:3])
            nc.vector.tensor_sub(var, var, m2)
            sc2 = stats.tile([1, 2], F32, tag="sc2")
            nc.vector.tensor_copy(out=sc2[:, 0:1], in_=e3[:, 0:1])
            std = stats.tile([1, 1], F32, tag="std")
            nc.scalar.activation(out=std, in_=var, func=ACTF.Sqrt,
                                 bias=eps_sb, scale=1.0)
            nc.vector.reciprocal(out=sc2[:, 1:2], in_=std)
            bc_ps = psmm.tile([P, 2], F32, tag="psmm")
            nc.tensor.matmul(bc_ps, lhsT=ones_row, rhs=sc2,
                             start=True, stop=True)
            # s = gamma * rstd ; t = beta - mean_total * s
            s_ch = stats.tile([P, KC], F32, tag="s_ch")
            nc.vector.tensor_scalar_mul(s_ch, gamma_sb, bc_ps[:, 1:2])
            t_ch = stats.tile([P, KC], F32, tag="t_ch")
            nc.vector.tensor_scalar_mul(t_ch, s_ch, bc_ps[:, 0:1])
            nc.vector.tensor_sub(t_ch, beta_sb, t_ch)
            st_scale[b] = (s_ch, t_ch)

        def emit_tail_apply(b):
            """normalization apply + writeout."""
            y_lo, y_hi = st_y[b]
            s_ch, t_ch = st_scale[b]
            for m in range(KC):
                yt = y_lo if m < 2 else y_hi
                mi = m % 2
                for h in range(2):
                    sl = slice(h * (N // 2), (h + 1) * (N // 2))
                    if b == 0 or m % 2 == 0:
                        nc.vector.tensor_scalar(
                            out=yt[:, mi, sl], in0=yt[:, mi, sl],
                            scalar1=s_ch[:, m:m + 1], scalar2=t_ch[:, m:m + 1],
                            op0=ALU.mult, op1=ALU.add)
                    else:
                        nc.scalar.activation(
                            out=yt[:, mi, sl], in_=yt[:, mi, sl],
                            func=ACTF.Identity,
                            bias=t_ch[:, m:m + 1], scale=s_ch[:, m:m + 1])
                    nc.sync.dma_start(out=out_d[b, m * P:(m + 1) * P, sl],
                                      in_=yt[:, mi, sl])

        emit_A_setup(0)
        emit_A_blocks(0, range(NB))
        emit_softmax(0)
        emit_A_setup(1)
        emit_A_blocks(1, range(2))
        emit_W2(0)
        emit_By(0)
        emit_A_blocks(1, range(2, NB))
        emit_softmax(1)
        emit_tail_stats(0)
        emit_W2(1)
        emit_tail_apply(0)
        emit_By(1)
        emit_tail_stats(1)
        emit_tail_apply(1)

    nc.finalize()
    return nc


_NC_CACHE = {}


def _get_nc():
    if "nc" not in _NC_CACHE:
        _NC_CACHE["nc"] = build_nc()
    return _NC_CACHE["nc"]


def _prep_w(w):
    # [C_in, C_out] -> [128, KC, C_out] fp16 with c_in = k*128 + p
    return np.ascontiguousarray(
        w.reshape(KC, P, C).transpose(1, 0, 2).astype(np.float16))


def _prep_vec(v):
    # [C] -> [128, KC] with c = k*128 + p
    return np.ascontiguousarray(v.reshape(KC, P).T)


def _prep_x(x):
    # [B, C, N] -> [B, NB, P, KC*512] fp16: block j, partition p, (k, n)
    nb = np.asarray(x).shape[0]
    xr = np.asarray(x, dtype=np.float32).reshape(nb, KC, P, NB, 512)
    return np.ascontiguousarray(
        xr.transpose(0, 3, 2, 1, 4).astype(np.float16)).reshape(
        nb, NB, P, KC * 512)


def _prep_x_local(x):
    return _prep_x(x)


def _make_in_maps(x, w_qkv, w_out, b_out, gamma, beta):
    xr = _prep_x(x)
    w_qkv = np.asarray(w_qkv, dtype=np.float32)
    wq = _prep_w(np.ascontiguousarray(w_qkv[0:C].T))
    wk = _prep_w(np.ascontiguousarray(w_qkv[C:2 * C].T))
    wv = _prep_w(np.ascontiguousarray(w_qkv[2 * C:3 * C].T))
    wo = _prep_w(np.ascontiguousarray(np.asarray(w_out, dtype=np.float32).T))
    bvec = _prep_vec(np.asarray(b_out, dtype=np.float32))
    gam = _prep_vec(np.asarray(gamma, dtype=np.float32))
    bet = _prep_vec(np.asarray(beta, dtype=np.float32))
    return [
        dict(x=np.ascontiguousarray(xr[c * PB:(c + 1) * PB]),
             wq=wq, wk=wk, wv=wv, wo=wo,
             bvec=bvec, gamma=gam, beta=bet)
        for c in range(NCORES)
    ]


def _run(inputs, trace=False, trace_kwargs=None):
    nc = _get_nc()
    in_maps = _make_in_maps(**inputs)
    res = run_bass_kernel_spmd(nc, in_maps, core_ids=list(range(NCORES)),
                               trace=trace, **(trace_kwargs or {}))
    out = np.concatenate([res.results[c]["out"].astype(np.float32)
                          for c in range(NCORES)], axis=0)
    return out.reshape(B, C, HW_SIDE, HW_SIDE), res


def kernel(x, w_qkv, w_out, b_out, gamma, beta):
    out, _ = _run(dict(x=x, w_qkv=w_qkv, w_out=w_out, b_out=b_out,
                       gamma=gamma, beta=beta))
    return out
